# revision 46
# baseline (speedup 1.0000x reference)
"""Trainium2 Bass kernel for the DPRNN block (channel-norm -> unfold ->
4x bidirectional SRU -> conv-transpose -> residual).

Sharding: data-parallel over the B*T=512 sequences; 64 sequences per core.
All weights replicated. Each core runs the full pipeline on its shard.

Layout (per core): sequences live in 128-column blocks (121 valid SRU steps
+ 7 pad columns). Pads carry f=0, b=0 through the scan so a single
tensor_tensor_scan over the whole free dim handles all sequences.

v3 over v2:
- norm: bf16 input, A/B scale-offset rows broadcast via DMA (no gpsimd
  partition_broadcast, no fp32 xn tile, no scalar-engine chunk copies);
  residual comes from the bf16 xn2 tile.
- SRU: f-gate weights negated host-side so both sigmoids run at
  scale=+1; f/r matmuls issued first so sigmoids start earlier;
  per-(o,ct) matmul ordering halves LDWEIGHTS pressure.
- conv: column-tiled matmuls (two 64-wide output tiles run concurrently
  in the PE array), residual add uses the shifted bf16 xn2 rows.
"""
import os
import numpy as np
import ml_dtypes

import concourse.bass as bass
import concourse.mybir as mybir
import concourse.tile as tile
from concourse import bacc
from concourse import bass_utils

F32 = mybir.dt.float32
BF16 = mybir.dt.bfloat16
FP8 = mybir.dt.float8e4

B, C, T, F_ = 4, 64, 128, 128
H, K = 128, 8
L = F_ - K + 1            # 121
EPS = 1e-8
NCORES = 8
NLOC = (B * T) // NCORES  # 64 sequences per core
NF = NLOC * 128           # 8192
XCOLS = NF + 8            # xn2 / h tiles carry 8 extra cols for shifted reads

DT_H = BF16       # h / xn2 / gate dtype (matmul inputs)
SPAN = 1024       # psum evacuation span (8 seqs)
NSPAN = NF // SPAN
HSPAN = NSPAN // 2
XC8 = NF + 16     # fp8 xn plane stride (16B-aligned for DoubleRow)
W0S = 16.0        # host-side fp8 weight scale for layer 0

_CACHE = {}


def _build():
    nc = bacc.Bacc("TRN2", target_bir_lowering=False, debug=False)
    AF = mybir.ActivationFunctionType
    OP = mybir.AluOpType

    # ---------------- DRAM tensors ----------------
    u_d = nc.dram_tensor("u", [C, NLOC, F_], BF16, kind="ExternalInput").ap()
    w0_d = nc.dram_tensor("w0f8", [2, 2, 128, 2, 512], FP8, kind="ExternalInput").ap()
    wi_d = nc.dram_tensor("wip", [3, 2, 2, 128, 512], BF16, kind="ExternalInput").ap()
    cw_d = nc.dram_tensor("cwp", [2, 8, 128, 64], BF16, kind="ExternalInput").ap()
    bf_d = nc.dram_tensor("bfp", [4, 2, 128], F32, kind="ExternalInput").ap()
    br_d = nc.dram_tensor("brp", [4, 2, 128], F32, kind="ExternalInput").ap()
    gm_d = nc.dram_tensor("gm", [C], F32, kind="ExternalInput").ap()
    bt_d = nc.dram_tensor("bt", [C], F32, kind="ExternalInput").ap()
    cb_d = nc.dram_tensor("cb", [128], F32, kind="ExternalInput").ap()
    out_d = nc.dram_tensor("o", [C, NF], F32, kind="ExternalOutput").ap()

    with tile.TileContext(nc) as tc:
        with tc.tile_pool(name="const", bufs=1) as cp:
            # ---- weights / biases resident in SBUF ----
            bfp_t = cp.tile([128, 8], F32)
            nc.sync.dma_start(bfp_t[:].rearrange("p (i d) -> p i d", i=4), bf_d.rearrange("i d p -> p i d"))
            brp_t = cp.tile([128, 8], F32)
            nc.sync.dma_start(brp_t[:].rearrange("p (i d) -> p i d", i=4), br_d.rearrange("i d p -> p i d"))
            cb_t = cp.tile([128, 1], F32)   # convb duplicated over both halves
            nc.sync.dma_start(cb_t[:], cb_d.rearrange("(c a) -> c a", a=1))

            # ---- long-lived activations ----
            xn2_t = cp.tile([128, XCOLS], DT_H)   # [xn ; xn shifted by 1] bf16
            h_t = [cp.tile([128, XCOLS], DT_H, name=f"h{i}") for i in range(4)]  # ping-pong pairs
            # fp8 copy of xn2 for the DoubleRow layer-0 matmuls: plane 0 is
            # xn2, plane 1 is xn2 shifted by 2 cols (the second 128-row
            # contraction half, i.e. taps +2/+3 of each weight chunk).
            xn8_t = cp.tile([128, 2 * XC8], FP8)
            xn8_v = xn8_t[:].rearrange("p (e x) -> p e x", e=2)

            nc.gpsimd.memset(xn2_t[:, NF:XCOLS], 0.0)
            nc.gpsimd.memset(xn2_t[64:128, NF - 1:NF], 0.0)
            nc.gpsimd.memset(xn8_v[:, :, NF:XC8], 0.0)

            # tiles only — the weight DMAs are issued after the u DMA below
            # so the stats input gets the bandwidth head start (w0 is needed
            # ~90us in, wi ~150us in).
            w0_t = cp.tile([128, 2 * 2 * 2 * 512], FP8)
            w0_v = w0_t[:].rearrange("p (d cp2 ko m) -> p d cp2 ko m", d=2, cp2=2, ko=2)
            wi_t = cp.tile([128, 3 * 2 * 2 * 512], BF16)
            wi_v = wi_t[:].rearrange("p (i d ct m) -> p i d ct m", i=3, d=2, ct=2)
            cw_t = cp.tile([128, 2 * 8 * 64], BF16)
            cw_v = cw_t[:].rearrange("p (ct k m) -> p ct k m", ct=2, k=8)

            # ================= channel norm =================
            # stats per (n, f) over c via matmul with a [128, 2] ones lhsT:
            # psum row 0 = mean(u), row 1 = mean(u^2). Scale/offset rows
            # A = rsqrt(var+eps), B = -mu*A are broadcast to the 64 channel
            # partitions by rank-1 matmuls with gamma/beta as the lhsT
            # column, folding the affine into the broadcast:
            #   psumA = gamma (x) A ; psumB = gamma (x) B + beta (x) 1.
            with (
                tc.tile_pool(name="normu", bufs=1) as np_,
                tc.tile_pool(name="normst", bufs=2, space="PSUM") as nst,
                tc.tile_pool(name="normbc", bufs=1, space="PSUM") as nbc,
                tc.tile_pool(name="normab", bufs=2) as nab,
            ):
                usq = np_.tile([128, NF], BF16)    # 0:64 u, 64:128 u^2
                # group 1 (seqs 32:64) first: its stats/apply feed L0 half 1
                nc.sync.dma_start(usq[0:64, NF // 2:NF],
                                  u_d[:, NLOC // 2:NLOC, :].rearrange("c n f -> c (n f)"))
                nc.sync.dma_start(usq[0:64, 0:NF // 2],
                                  u_d[:, 0:NLOC // 2, :].rearrange("c n f -> c (n f)"))
                # weight DMAs queue behind u
                nc.scalar.dma_start(w0_v, w0_d.rearrange("d cp2 p ko m -> p d cp2 ko m"))
                nc.scalar.dma_start(cw_v, cw_d.rearrange("ct k p m -> p ct k m"))
                nc.scalar.dma_start(wi_v, wi_d.rearrange("i d ct p m -> p i d ct m"))
                # PE warmup: HAM un-throttles (1.2 -> 2.4 GHz) only after
                # ~3.4us of sustained matmul activity. Burn dummy matmuls on
                # scratch data while the u DMA is in flight (kept short so
                # the stats matmuls aren't stuck behind them in the PE FIFO).
                scr = np_.tile([128, 512], BF16)
                nc.vector.memset(scr[:], 0.0)
                for wi_ in range(24):
                    wm = nst.tile([2, 512], F32, tag="warm")
                    nc.tensor.matmul(wm[:], scr[:, 0:2], scr[:], start=True, stop=True)
                ones2 = np_.tile([128, 2], BF16)
                nc.vector.memset(ones2[:], 0.0)
                nc.vector.memset(ones2[0:64, 0:1], 1.0 / C)
                nc.vector.memset(ones2[64:128, 1:2], 1.0 / C)
                stf = np_.tile([2, 1024], F32)      # stats staging (mu|s2 rows)
                stT = np_.tile([NLOC, 256], F32)    # rows n: cols 0:128 mu, 128:256 s2
                A_t = np_.tile([NLOC, 128], BF16)   # rstd (per seq-row, per f)
                B_t = np_.tile([NLOC, 128], BF16)   # -mu*rstd
                sc1 = np_.tile([NLOC, 128], F32)
                eps_t = np_.tile([NLOC, 1], F32)
                nc.vector.memset(eps_t[:], EPS)
                gmf = np_.tile([1, C], F32)
                nc.sync.dma_start(gmf[:], gm_d.rearrange("(a c) -> a c", a=1))
                btf = np_.tile([1, C], F32)
                nc.sync.dma_start(btf[:], bt_d.rearrange("(a c) -> a c", a=1))
                gmr = np_.tile([1, C], BF16)        # gamma as a lhsT row
                nc.vector.tensor_copy(gmr[:], gmf[:])
                btr = np_.tile([1, C], BF16)        # beta as a lhsT row
                nc.vector.tensor_copy(btr[:], btf[:])
                one5 = np_.tile([1, 512], BF16)
                nc.vector.memset(one5[:], 1.0)

                CH = 1024
                # stats for BOTH groups first: group-0 stats (PE/ACT) then
                # overlap group-1's applies (DVE) instead of queueing after
                # them on the in-order engines.
                for g in (1, 0):
                    gsl = slice(g * (NF // 2), (g + 1) * (NF // 2))
                    nr = slice(g * 32, g * 32 + 32)
                    nc.scalar.activation(usq[64:128, gsl], usq[0:64, gsl],
                                         AF.Square, bias=0.0)
                    for j in range(8):
                        st_ps = nst.tile([2, 512], F32, tag="st")
                        nc.tensor.matmul(st_ps[:], ones2[:],
                                         usq[:, g * 4096 + j * 512:g * 4096 + (j + 1) * 512],
                                         start=True, stop=True)
                        nc.scalar.copy(stf[:, (j % 2) * 512:(j % 2) * 512 + 512], st_ps[:])
                        if j % 2 == 1:
                            # transpose: row 0 (mu) / row 1 (s2) -> [8 rows, 128]
                            nq = slice(g * 32 + (j // 2) * 8, g * 32 + (j // 2) * 8 + 8)
                            nc.sync.dma_start(stT[nq, 0:128], stf[0:1, :])
                            nc.sync.dma_start(stT[nq, 128:256], stf[1:2, :])
                    # dummy matmul chained on the transpose keeps HAM warm
                    wm = nst.tile([2, 512], F32, tag="warm")
                    nc.tensor.matmul(wm[:, 0:256], stT[nr, 0:2], stT[nr, :],
                                     start=True, stop=True)
                    mu_v = stT[nr, 0:128]
                    s2_v = stT[nr, 128:256]
                    nc.vector.tensor_mul(sc1[nr, :], mu_v, mu_v)
                    nc.vector.tensor_sub(s2_v, s2_v, sc1[nr, :])   # var
                    nc.scalar.activation(sc1[nr, :], s2_v, AF.Sqrt, bias=eps_t[nr, 0:1])
                    with nc.allow_low_precision(reason="rstd rounded to bf16 for the bf16 broadcast matmul"):
                        nc.vector.reciprocal(A_t[nr, :], sc1[nr, :])   # rstd
                    nc.vector.scalar_tensor_tensor(
                        B_t[nr, :], mu_v, -1.0, A_t[nr, :], op0=OP.mult, op1=OP.mult
                    )
                    wm = nst.tile([2, 512], F32, tag="warm")
                    nc.tensor.matmul(wm[:, 0:128], stT[nr, 0:2], stT[nr, 0:128],
                                     start=True, stop=True)
                # broadcast A|B chunk rows to the 64 channel partitions as
                # rank-1 matmuls with gamma/beta lhsT columns (folds the
                # affine), then xn = psumA*u + psumB on DVE.
                for ch in range(7, -1, -1):
                    rs = slice(ch * 8, ch * 8 + 8)
                    ab1 = nab.tile([1, 2 * CH], BF16, tag="ab1")
                    nc.sync.dma_start(ab1[:, 0:CH], A_t[rs, :])
                    nc.sync.dma_start(ab1[:, CH:2 * CH], B_t[rs, :])
                    abP = nbc.tile([64, 2 * CH], F32, tag="ab")
                    for s5 in range(2):
                        q = slice(s5 * 512, s5 * 512 + 512)
                        nc.tensor.matmul(abP[:, q], gmr[:], ab1[:, q],
                                         start=True, stop=True)
                        q2 = slice(CH + s5 * 512, CH + s5 * 512 + 512)
                        nc.tensor.matmul(abP[:, q2], gmr[:], ab1[:, q2],
                                         start=True, stop=False)
                        nc.tensor.matmul(abP[:, q2], btr[:], one5[:],
                                         start=False, stop=True)
                    sl = slice(ch * CH, (ch + 1) * CH)
                    nc.vector.tensor_mul(xn2_t[0:64, sl], usq[0:64, sl], abP[:, 0:CH])
                    nc.vector.tensor_add(xn2_t[0:64, sl], xn2_t[0:64, sl], abP[:, CH:2 * CH])
                    # bf16 shifted copy into rows 64:128 (chunk ch reads
                    # chunk ch+1's first col, already written because
                    # chunks go in reverse order)
                    hi = min((ch + 1) * CH + 1, NF)
                    nc.scalar.copy(
                        xn2_t[64:128, ch * CH:hi - 1],
                        xn2_t[0:64, ch * CH + 1:hi],
                    )
                    # fp8 planes for layer 0 (plane 1 reads 2 cols into
                    # the next chunk, already written in reverse order)
                    nc.scalar.copy(xn8_v[:, 0, sl], xn2_t[:, sl])
                    nc.vector.tensor_copy(
                        xn8_v[:, 1, sl],
                        xn2_t[:, ch * CH + 2:(ch + 1) * CH + 2],
                    )

            # ================= SRU layers =================
            sig = AF.Sigmoid
            with (
                tc.tile_pool(name="gates", bufs=2) as gp,
                tc.tile_pool(name="lps", bufs=1, space="PSUM") as pp,
            ):
                for li in range(4):
                    if li == 0:
                        hin = None
                        nct = 2   # two DoubleRow chunks of 256 contraction
                        psc = 1.0 / W0S
                    else:
                        hin = [h_t[2 * ((li - 1) % 2)], h_t[2 * ((li - 1) % 2) + 1]]
                        nct = 2
                        psc = 1.0
                    hout = [h_t[2 * (li % 2)], h_t[2 * (li % 2) + 1]]
                    ooff = 8 if li == 3 else 0
                    for half in (1, 0):
                        for d in range(2):
                            bcol = bfp_t[:, 2 * li + d:2 * li + d + 1]
                            rcol = brp_t[:, 2 * li + d:2 * li + d + 1]
                            f_t = gp.tile([128, NF // 2], DT_H, tag="f")  # g, then 1-g, then c-hp
                            r_t = gp.tile([128, NF // 2], DT_H, tag="r")
                            zw_t = gp.tile([128, NF], DT_H, tag="zw")     # z | hp
                            z_h = zw_t[:, 0:NF // 2]
                            w_h = zw_t[:, NF // 2:NF]
                            for s4 in range(HSPAN):
                                span = half * HSPAN + s4
                                fr_ps = pp.tile([128, 2 * SPAN], F32, name="fr", tag="fr")
                                zw_ps = pp.tile([128, 2 * SPAN], F32, name="zw", tag="zw")
                                # f/r matmuls first so the sigmoids start
                                # early; per-(o,ct) inner pairing reuses each
                                # weight for two 512-col matmuls.
                                pst = [(1, fr_ps[:, 0:SPAN]), (2, fr_ps[:, SPAN:2 * SPAN]),
                                       (0, zw_ps[:, 0:SPAN]), (3, zw_ps[:, SPAN:2 * SPAN])]
                                for o, dst in pst:
                                    for ct in range(nct):
                                        for h2 in range(SPAN // 512):
                                            osl = dst[:, h2 * 512:(h2 + 1) * 512]
                                            base = span * SPAN + h2 * 512
                                            if li == 0:
                                                lhsT = w0_v[:, d, ct, :, o * 128:(o + 1) * 128]
                                                rhs = xn8_v[:, :, base + 4 * ct:base + 4 * ct + 512]
                                                nc.tensor.matmul(
                                                    osl, lhsT, rhs,
                                                    start=(ct == 0), stop=(ct == nct - 1),
                                                    perf_mode=mybir.MatmulPerfMode.DoubleRow,
                                                )
                                            else:
                                                lhsT = wi_v[:, li - 1, d, ct, o * 128:(o + 1) * 128]
                                                rhs = hin[ct][:, base:base + 512]
                                                nc.tensor.matmul(
                                                    osl, lhsT, rhs,
                                                    start=(ct == 0), stop=(ct == nct - 1),
                                                )
                                # evacuate span. d=1 stores each 128-block
                                # reversed (pads land at l' in [0,7)).
                                ssl = slice(s4 * SPAN, (s4 + 1) * SPAN)
                                f_src = fr_ps[:, 0:SPAN].rearrange("p (n l) -> p n l", l=128)
                                r_src = fr_ps[:, SPAN:2 * SPAN].rearrange("p (n l) -> p n l", l=128)
                                zw_src = zw_ps[:].rearrange("p (w n l) -> p w n l", w=2, l=128)
                                if d == 1:
                                    f_src = f_src[:, :, ::-1]
                                    r_src = r_src[:, :, ::-1]
                                    zw_src = zw_src[:, :, :, ::-1]
                                # f-gate weights are negated host-side, so
                                # sigmoid(psum*psc + (-bf)) = 1 - f = g.
                                nc.scalar.activation(f_t[:, ssl], f_src, sig, bias=bcol, scale=psc)
                                nc.scalar.activation(r_t[:, ssl], r_src, sig, bias=rcol, scale=psc)
                                zw_dst = zw_t[:].rearrange("p (w x) -> p w x", w=2)[:, :, ssl]
                                if li == 0:
                                    # z|hp scaled back by 1/W0S during evac
                                    nc.scalar.activation(zw_dst, zw_src, AF.Copy, scale=psc)
                                else:
                                    nc.scalar.copy(zw_dst, zw_src)
                            # b = g*z (in place over z; read g before the 1-g
                            # pass below overwrites it)
                            nc.vector.tensor_mul(z_h, f_t[:], z_h)
                            # f = 1 - g
                            nc.vector.tensor_scalar(f_t[:], f_t[:], -1.0, 1.0,
                                                    op0=OP.mult, op1=OP.add)
                            # pads reset the scan carry between sequences
                            pads = slice(121, 128) if d == 0 else slice(0, 7)
                            f_v = f_t[:].rearrange("p (n l) -> p n l", l=128)
                            b_v = z_h.rearrange("p (n l) -> p n l", l=128)
                            nc.gpsimd.memset(f_v[:, :, pads], 0.0)
                            nc.gpsimd.memset(b_v[:, :, pads], 0.0)
                            # c = f*c + (1-f)*z
                            nc.vector.tensor_tensor_scan(
                                z_h, f_t[:], z_h, 0.0,
                                op0=OP.mult, op1=OP.add,
                            )
                            # highway: out = r*(cs-hp) + hp
                            nc.vector.tensor_sub(f_t[:], z_h, w_h)
                            nc.vector.tensor_mul(r_t[:], r_t[:], f_t[:])
                            hov = hout[d][:, ooff:ooff + NF].rearrange(
                                "p (n l) -> p n l", l=128
                            )
                            dst = hov[:, half * 32:half * 32 + 32, :]
                            if d == 1:
                                dst = dst[:, :, ::-1]
                            r_v = r_t[:].rearrange("p (n l) -> p n l", l=128)
                            w_v = w_h.rearrange("p (n l) -> p n l", l=128)
                            nc.vector.tensor_add(dst, r_v[:, :, :], w_v[:, :, :])

            # ================= transposed conv + residual =================
            h4 = [h_t[2], h_t[3]]  # layer 3 writes pair B at offset 8
            for t4 in h4:
                v = t4[:, 0:NF].rearrange("p (n l) -> p n l", l=128)
                nc.gpsimd.memset(t4[:, 0:8], 0.0)
                nc.gpsimd.memset(v[:, 1:33, 1:8], 0.0)
                nc.gpsimd.memset(v[:, 33:64, 1:8], 0.0)
                nc.gpsimd.memset(t4[:, NF + 1:XCOLS], 0.0)
            with (
                tc.tile_pool(name="cvp", bufs=4, space="PSUM") as cvp,
                tc.tile_pool(name="osp", bufs=4) as osp,
            ):
                # column-tiled: chunk pair (2j, 2j+1) runs concurrently in PE
                # col-groups 0:64 / 64:128. Reversed: the last pairs need
                # half-1 data (finished first), overlapping layer 3's tail.
                for pair in reversed(range(NF // 1024)):
                    cA = 2 * pair
                    cB = 2 * pair + 1
                    c_ps = cvp.tile([128, 512], F32, tag="c")
                    mm = 0
                    for ct in range(2):
                        for k in range(8):
                            bA = cA * 512 + 8 - k
                            bB = cB * 512 + 8 - k
                            nc.tensor.matmul(
                                c_ps[0:64, :], cw_v[:, ct, k, :], h4[ct][:, bA:bA + 512],
                                start=(mm == 0), stop=(mm == 15), tile_position=(0, 0),
                            )
                            nc.tensor.matmul(
                                c_ps[64:128, :], cw_v[:, ct, k, :], h4[ct][:, bB:bB + 512],
                                start=(mm == 0), stop=(mm == 15), tile_position=(0, 64),
                            )
                            mm += 1
                    o_t = osp.tile([128, 512], F32, tag="o")
                    slA = slice(cA * 512, cA * 512 + 512)
                    slB = slice(cB * 512, cB * 512 + 512)
                    nc.vector.scalar_tensor_tensor(
                        o_t[0:64, :], c_ps[0:64, :], cb_t[0:64, 0:1], xn2_t[0:64, slA],
                        op0=OP.add, op1=OP.add,
                    )
                    # rows 64:128 read the residual from the shifted xn2 rows
                    nc.vector.scalar_tensor_tensor(
                        o_t[64:128, :], c_ps[64:128, :], cb_t[64:128, 0:1],
                        xn2_t[64:128, cB * 512 - 1:cB * 512 + 511],
                        op0=OP.add, op1=OP.add,
                    )
                    nc.sync.dma_start(out_d[:, slA], o_t[0:64, :])
                    nc.sync.dma_start(out_d[:, slB], o_t[64:128, :])

    nc.compile()
    return nc


def _prep_weights(W0, Ws, convW):
    w0r = W0.reshape(C, K, 2, 4 * H)
    w0p = np.zeros((2, 4, 128, 512), np.float32)
    for d in range(2):
        for kp in range(4):
            w0p[d, kp, 0:64] = w0r[:, 2 * kp, d]
            w0p[d, kp, 64:128] = w0r[:, 2 * kp + 1, d]
    wip = np.zeros((3, 2, 2, 128, 512), np.float32)
    for i in range(3):
        for d in range(2):
            for ct in range(2):
                wip[i, d, ct] = Ws[i][ct * 128:(ct + 1) * 128, d]
    # negate the f-gate output chunk so sigmoid(psum - bf) = 1-f = g
    # without a scale=-1 activation.
    w0p[:, :, :, 128:256] *= -1.0
    wip[:, :, :, :, 128:256] *= -1.0
    # layer-0 weights as fp8 DoubleRow pairs, scaled by W0S to stay out of
    # the e4m3 subnormal range (compensated by psum scale 1/W0S on-chip):
    # w0f8[d, ctp, p, ko, m] = W0S * w0p[d, 2*ctp+ko, p, m]
    w0f8 = np.zeros((2, 2, 128, 2, 512), np.float32)
    for ctp in range(2):
        for ko in range(2):
            w0f8[:, ctp, :, ko, :] = W0S * w0p[:, 2 * ctp + ko].transpose(0, 1, 2)
    cwp = np.zeros((2, 8, 128, C), np.float32)
    for ct in range(2):
        for k in range(8):
            cwp[ct, k] = convW[ct * 128:(ct + 1) * 128, :, k]
    bf16 = ml_dtypes.bfloat16
    f8 = ml_dtypes.float8_e4m3
    return w0f8.astype(f8), wip.astype(bf16), cwp.astype(bf16)


def kernel(**inputs):
    inputs = {k: np.asarray(v) for k, v in inputs.items()}
    x = inputs["x"].astype(np.float32)
    xs = np.ascontiguousarray(
        x.transpose(0, 2, 1, 3).reshape(B * T, C, F_)
    )  # (512, C, F)

    w0f8, wip, cwp = _prep_weights(
        inputs["W0"].astype(np.float32),
        [inputs[f"W{i}"].astype(np.float32) for i in (1, 2, 3)],
        inputs["convW"].astype(np.float32),
    )
    bfp = -np.stack([inputs[f"bf{i}"] for i in range(4)]).astype(np.float32)
    brp = np.stack([inputs[f"br{i}"] for i in range(4)]).astype(np.float32)
    gm = inputs["gamma"].reshape(C).astype(np.float32)
    bt = inputs["beta"].reshape(C).astype(np.float32)
    cb = np.tile(inputs["convb"].reshape(C).astype(np.float32), 2)  # [128]

    if "nc" not in _CACHE:
        _CACHE["nc"] = _build()
    nc = _CACHE["nc"]

    bf16 = ml_dtypes.bfloat16
    shared = {"w0f8": w0f8, "wip": wip, "cwp": cwp, "bfp": bfp, "brp": brp,
              "gm": gm, "bt": bt, "cb": cb}
    in_maps = []
    for core in range(NCORES):
        sh = xs[core * NLOC:(core + 1) * NLOC]  # (NLOC, C, F)
        u = np.ascontiguousarray(sh.transpose(1, 0, 2)).astype(bf16)  # (C, NLOC, F)
        in_maps.append({"u": u, **shared})

    trace = bool(os.environ.get("KBENCH_TRACE"))
    res = bass_utils.run_bass_kernel_spmd(
        nc, in_maps, list(range(NCORES)), trace=trace,
        tmpdir=os.environ.get("KBENCH_TMPDIR"),
    )
    _CACHE["last_result"] = res

    full = np.concatenate(
        [res.results[i]["o"].reshape(C, NLOC, F_) for i in range(NCORES)], axis=1
    )  # (C, 512, F)
    out = full.transpose(1, 0, 2).reshape(B, T, C, F_).transpose(0, 2, 1, 3)
    return np.ascontiguousarray(out.astype(np.float32))


# revision 50
# speedup vs baseline: 1.0505x; 1.0505x over previous
"""Trainium2 Bass kernel for the DPRNN block (channel-norm -> unfold ->
4x bidirectional SRU -> conv-transpose -> residual).

Sharding: data-parallel over the B*T=512 sequences; 64 sequences per core.
All weights replicated. Each core runs the full pipeline on its shard.

Layout (per core): sequences live in 128-column blocks (121 valid SRU steps
+ 7 pad columns). Pads carry f=0, b=0 through the scan so a single
tensor_tensor_scan over the whole free dim handles all sequences.

v3 over v2:
- norm: bf16 input, A/B scale-offset rows broadcast via DMA (no gpsimd
  partition_broadcast, no fp32 xn tile, no scalar-engine chunk copies);
  residual comes from the bf16 xn2 tile.
- SRU: f-gate weights negated host-side so both sigmoids run at
  scale=+1; f/r matmuls issued first so sigmoids start earlier;
  per-(o,ct) matmul ordering halves LDWEIGHTS pressure.
- conv: column-tiled matmuls (two 64-wide output tiles run concurrently
  in the PE array), residual add uses the shifted bf16 xn2 rows.
"""
import os
import numpy as np
import ml_dtypes

import concourse.bass as bass
import concourse.mybir as mybir
import concourse.tile as tile
from concourse import bacc
from concourse import bass_utils

F32 = mybir.dt.float32
BF16 = mybir.dt.bfloat16
FP8 = mybir.dt.float8e4

B, C, T, F_ = 4, 64, 128, 128
H, K = 128, 8
L = F_ - K + 1            # 121
EPS = 1e-8
NCORES = 8
NLOC = (B * T) // NCORES  # 64 sequences per core
NF = NLOC * 128           # 8192
XCOLS = NF + 8            # xn2 / h tiles carry 8 extra cols for shifted reads

DT_H = BF16       # h / xn2 / gate dtype (matmul inputs)
SPAN = 1024       # psum evacuation span (8 seqs)
NSPAN = NF // SPAN
HSPAN = NSPAN // 2
XC8 = NF + 16     # fp8 xn plane stride (16B-aligned for DoubleRow)
W0S = 16.0        # host-side fp8 weight scale for layer 0

_CACHE = {}


def _build():
    nc = bacc.Bacc("TRN2", target_bir_lowering=False, debug=False)
    AF = mybir.ActivationFunctionType
    OP = mybir.AluOpType

    # ---------------- DRAM tensors ----------------
    u_d = nc.dram_tensor("u", [C, NLOC, F_], BF16, kind="ExternalInput").ap()
    w0_d = nc.dram_tensor("w0f8", [2, 2, 128, 2, 512], FP8, kind="ExternalInput").ap()
    wi_d = nc.dram_tensor("wip", [3, 2, 2, 128, 512], BF16, kind="ExternalInput").ap()
    cw_d = nc.dram_tensor("cwp", [2, 8, 128, 64], BF16, kind="ExternalInput").ap()
    bf_d = nc.dram_tensor("bfp", [4, 2, 128], F32, kind="ExternalInput").ap()
    br_d = nc.dram_tensor("brp", [4, 2, 128], F32, kind="ExternalInput").ap()
    gm_d = nc.dram_tensor("gm", [C], F32, kind="ExternalInput").ap()
    bt_d = nc.dram_tensor("bt", [C], F32, kind="ExternalInput").ap()
    cb_d = nc.dram_tensor("cb", [128], F32, kind="ExternalInput").ap()
    out_d = nc.dram_tensor("o", [C, NF], BF16, kind="ExternalOutput").ap()

    with tile.TileContext(nc) as tc:
        with tc.tile_pool(name="const", bufs=1) as cp:
            # ---- weights / biases resident in SBUF ----
            bfp_t = cp.tile([128, 8], F32)
            nc.sync.dma_start(bfp_t[:].rearrange("p (i d) -> p i d", i=4), bf_d.rearrange("i d p -> p i d"))
            brp_t = cp.tile([128, 8], F32)
            nc.sync.dma_start(brp_t[:].rearrange("p (i d) -> p i d", i=4), br_d.rearrange("i d p -> p i d"))
            cb_t = cp.tile([128, 1], F32)   # convb duplicated over both halves
            nc.sync.dma_start(cb_t[:], cb_d.rearrange("(c a) -> c a", a=1))

            # ---- long-lived activations ----
            xn2_t = cp.tile([128, XCOLS], DT_H)   # [xn ; xn shifted by 1] bf16
            h_t = [cp.tile([128, XCOLS], DT_H, name=f"h{i}") for i in range(4)]  # ping-pong pairs
            # fp8 copy of xn2 for the DoubleRow layer-0 matmuls: plane 0 is
            # xn2, plane 1 is xn2 shifted by 2 cols (the second 128-row
            # contraction half, i.e. taps +2/+3 of each weight chunk).
            xn8_t = cp.tile([128, 2 * XC8], FP8)
            xn8_v = xn8_t[:].rearrange("p (e x) -> p e x", e=2)

            nc.gpsimd.memset(xn2_t[:, NF:XCOLS], 0.0)
            nc.gpsimd.memset(xn2_t[64:128, NF - 1:NF], 0.0)
            nc.gpsimd.memset(xn8_v[:, :, NF:XC8], 0.0)

            # tiles only — the weight DMAs are issued after the u DMA below
            # so the stats input gets the bandwidth head start (w0 is needed
            # ~90us in, wi ~150us in).
            w0_t = cp.tile([128, 2 * 2 * 2 * 512], FP8)
            w0_v = w0_t[:].rearrange("p (d cp2 ko m) -> p d cp2 ko m", d=2, cp2=2, ko=2)
            wi_t = cp.tile([128, 3 * 2 * 2 * 512], BF16)
            wi_v = wi_t[:].rearrange("p (i d ct m) -> p i d ct m", i=3, d=2, ct=2)
            cw_t = cp.tile([128, 2 * 8 * 64], BF16)
            cw_v = cw_t[:].rearrange("p (ct k m) -> p ct k m", ct=2, k=8)

            # ================= channel norm =================
            # stats per (n, f) over c via matmul with a [128, 2] ones lhsT:
            # psum row 0 = mean(u), row 1 = mean(u^2). Scale/offset rows
            # A = rsqrt(var+eps), B = -mu*A are broadcast to the 64 channel
            # partitions by rank-1 matmuls with gamma/beta as the lhsT
            # column, folding the affine into the broadcast:
            #   psumA = gamma (x) A ; psumB = gamma (x) B + beta (x) 1.
            with (
                tc.tile_pool(name="normu", bufs=1) as np_,
                tc.tile_pool(name="normst", bufs=2, space="PSUM") as nst,
                tc.tile_pool(name="normbc", bufs=1, space="PSUM") as nbc,
                tc.tile_pool(name="normab", bufs=2) as nab,
            ):
                usq = np_.tile([128, NF], BF16)    # 0:64 u, 64:128 u^2
                # group 1 (seqs 32:64) first: its stats/apply feed L0 half 1
                nc.sync.dma_start(usq[0:64, NF // 2:NF],
                                  u_d[:, NLOC // 2:NLOC, :].rearrange("c n f -> c (n f)"))
                nc.sync.dma_start(usq[0:64, 0:NF // 2],
                                  u_d[:, 0:NLOC // 2, :].rearrange("c n f -> c (n f)"))
                # weight DMAs queue behind u
                nc.scalar.dma_start(w0_v, w0_d.rearrange("d cp2 p ko m -> p d cp2 ko m"))
                nc.scalar.dma_start(cw_v, cw_d.rearrange("ct k p m -> p ct k m"))
                nc.scalar.dma_start(wi_v, wi_d.rearrange("i d ct p m -> p i d ct m"))
                # PE warmup: HAM un-throttles (1.2 -> 2.4 GHz) only after
                # ~3.4us of sustained matmul activity. Burn dummy matmuls on
                # scratch data while the u DMA is in flight (kept short so
                # the stats matmuls aren't stuck behind them in the PE FIFO).
                scr = np_.tile([128, 512], BF16)
                nc.vector.memset(scr[:], 0.0)
                for wi_ in range(24):
                    wm = nst.tile([2, 512], F32, tag="warm")
                    nc.tensor.matmul(wm[:], scr[:, 0:2], scr[:], start=True, stop=True)
                ones2 = np_.tile([128, 2], BF16)
                nc.vector.memset(ones2[:], 0.0)
                nc.vector.memset(ones2[0:64, 0:1], 1.0 / C)
                nc.vector.memset(ones2[64:128, 1:2], 1.0 / C)
                stf = np_.tile([2, 1024], F32)      # stats staging (mu|s2 rows)
                stT = np_.tile([NLOC, 256], F32)    # rows n: cols 0:128 mu, 128:256 s2
                A_t = np_.tile([NLOC, 128], BF16)   # rstd (per seq-row, per f)
                B_t = np_.tile([NLOC, 128], BF16)   # -mu*rstd
                sc1 = np_.tile([NLOC, 128], F32)
                eps_t = np_.tile([NLOC, 1], F32)
                nc.vector.memset(eps_t[:], EPS)
                gmf = np_.tile([1, C], F32)
                nc.sync.dma_start(gmf[:], gm_d.rearrange("(a c) -> a c", a=1))
                btf = np_.tile([1, C], F32)
                nc.sync.dma_start(btf[:], bt_d.rearrange("(a c) -> a c", a=1))
                gmr = np_.tile([1, C], BF16)        # gamma as a lhsT row
                nc.vector.tensor_copy(gmr[:], gmf[:])
                btr = np_.tile([1, C], BF16)        # beta as a lhsT row
                nc.vector.tensor_copy(btr[:], btf[:])
                one5 = np_.tile([1, 512], BF16)
                nc.vector.memset(one5[:], 1.0)

                CH = 1024
                # stats for BOTH groups first: group-0 stats (PE/ACT) then
                # overlap group-1's applies (DVE) instead of queueing after
                # them on the in-order engines.
                for g in (1, 0):
                    gsl = slice(g * (NF // 2), (g + 1) * (NF // 2))
                    nr = slice(g * 32, g * 32 + 32)
                    nc.scalar.activation(usq[64:128, gsl], usq[0:64, gsl],
                                         AF.Square, bias=0.0)
                    for j in range(8):
                        st_ps = nst.tile([2, 512], F32, tag="st")
                        nc.tensor.matmul(st_ps[:], ones2[:],
                                         usq[:, g * 4096 + j * 512:g * 4096 + (j + 1) * 512],
                                         start=True, stop=True)
                        nc.scalar.copy(stf[:, (j % 2) * 512:(j % 2) * 512 + 512], st_ps[:])
                        if j % 2 == 1:
                            # transpose: row 0 (mu) / row 1 (s2) -> [8 rows, 128]
                            nq = slice(g * 32 + (j // 2) * 8, g * 32 + (j // 2) * 8 + 8)
                            nc.sync.dma_start(stT[nq, 0:128], stf[0:1, :])
                            nc.sync.dma_start(stT[nq, 128:256], stf[1:2, :])
                    # dummy matmul chained on the transpose keeps HAM warm
                    wm = nst.tile([2, 512], F32, tag="warm")
                    nc.tensor.matmul(wm[:, 0:256], stT[nr, 0:2], stT[nr, :],
                                     start=True, stop=True)
                    mu_v = stT[nr, 0:128]
                    s2_v = stT[nr, 128:256]
                    nc.vector.tensor_mul(sc1[nr, :], mu_v, mu_v)
                    nc.vector.tensor_sub(s2_v, s2_v, sc1[nr, :])   # var
                    nc.scalar.activation(sc1[nr, :], s2_v, AF.Sqrt, bias=eps_t[nr, 0:1])
                    with nc.allow_low_precision(reason="rstd rounded to bf16 for the bf16 broadcast matmul"):
                        nc.vector.reciprocal(A_t[nr, :], sc1[nr, :])   # rstd
                    nc.vector.scalar_tensor_tensor(
                        B_t[nr, :], mu_v, -1.0, A_t[nr, :], op0=OP.mult, op1=OP.mult
                    )
                    wm = nst.tile([2, 512], F32, tag="warm")
                    nc.tensor.matmul(wm[:, 0:128], stT[nr, 0:2], stT[nr, 0:128],
                                     start=True, stop=True)
                # broadcast A|B chunk rows to the 64 channel partitions as
                # rank-1 matmuls with gamma/beta lhsT columns (folds the
                # affine), then xn = psumA*u + psumB on DVE.
                for ch in range(7, -1, -1):
                    rs = slice(ch * 8, ch * 8 + 8)
                    ab1 = nab.tile([1, 2 * CH], BF16, tag="ab1")
                    nc.sync.dma_start(ab1[:, 0:CH], A_t[rs, :])
                    nc.sync.dma_start(ab1[:, CH:2 * CH], B_t[rs, :])
                    abP = nbc.tile([64, 2 * CH], F32, tag="ab")
                    for s5 in range(2):
                        q = slice(s5 * 512, s5 * 512 + 512)
                        nc.tensor.matmul(abP[:, q], gmr[:], ab1[:, q],
                                         start=True, stop=True)
                        q2 = slice(CH + s5 * 512, CH + s5 * 512 + 512)
                        nc.tensor.matmul(abP[:, q2], gmr[:], ab1[:, q2],
                                         start=True, stop=False)
                        nc.tensor.matmul(abP[:, q2], btr[:], one5[:],
                                         start=False, stop=True)
                    sl = slice(ch * CH, (ch + 1) * CH)
                    nc.vector.tensor_mul(xn2_t[0:64, sl], usq[0:64, sl], abP[:, 0:CH])
                    nc.vector.tensor_add(xn2_t[0:64, sl], xn2_t[0:64, sl], abP[:, CH:2 * CH])
                    # bf16 shifted copy into rows 64:128 (chunk ch reads
                    # chunk ch+1's first col, already written because
                    # chunks go in reverse order)
                    hi = min((ch + 1) * CH + 1, NF)
                    nc.scalar.copy(
                        xn2_t[64:128, ch * CH:hi - 1],
                        xn2_t[0:64, ch * CH + 1:hi],
                    )
                    # fp8 planes for layer 0 (plane 1 reads 2 cols into
                    # the next chunk, already written in reverse order)
                    nc.scalar.copy(xn8_v[:, 0, sl], xn2_t[:, sl])
                    nc.vector.tensor_copy(
                        xn8_v[:, 1, sl],
                        xn2_t[:, ch * CH + 2:(ch + 1) * CH + 2],
                    )

            # ================= SRU layers =================
            sig = AF.Sigmoid
            with (
                tc.tile_pool(name="gates", bufs=2) as gp,
                tc.tile_pool(name="lps", bufs=1, space="PSUM") as pp,
            ):
                for li in range(4):
                    if li == 0:
                        hin = None
                        nct = 2   # two DoubleRow chunks of 256 contraction
                        psc = 1.0 / W0S
                    else:
                        hin = [h_t[2 * ((li - 1) % 2)], h_t[2 * ((li - 1) % 2) + 1]]
                        nct = 2
                        psc = 1.0
                    hout = [h_t[2 * (li % 2)], h_t[2 * (li % 2) + 1]]
                    ooff = 8 if li == 3 else 0
                    for half in (1, 0):
                        for d in range(2):
                            bcol = bfp_t[:, 2 * li + d:2 * li + d + 1]
                            rcol = brp_t[:, 2 * li + d:2 * li + d + 1]
                            f_t = gp.tile([128, NF // 2], DT_H, tag="f")  # g, then 1-g, then c-hp
                            r_t = gp.tile([128, NF // 2], DT_H, tag="r")
                            z_t = gp.tile([128, NF // 2], DT_H, tag="z")  # z, then b, then c
                            w_t = gp.tile([128, NF // 2], DT_H, tag="w")  # hp
                            z_h = z_t[:]
                            w_h = w_t[:]
                            for s4 in range(HSPAN):
                                span = half * HSPAN + s4
                                fr_ps = pp.tile([128, 2 * SPAN], F32, name="fr", tag="fr")
                                zw_ps = pp.tile([128, 2 * SPAN], F32, name="zw", tag="zw")
                                # f/r matmuls first so the sigmoids start
                                # early; per-(o,ct) inner pairing reuses each
                                # weight for two 512-col matmuls.
                                pst = [(1, fr_ps[:, 0:SPAN]), (2, fr_ps[:, SPAN:2 * SPAN]),
                                       (0, zw_ps[:, 0:SPAN]), (3, zw_ps[:, SPAN:2 * SPAN])]
                                for o, dst in pst:
                                    for ct in range(nct):
                                        for h2 in range(SPAN // 512):
                                            osl = dst[:, h2 * 512:(h2 + 1) * 512]
                                            base = span * SPAN + h2 * 512
                                            if li == 0:
                                                lhsT = w0_v[:, d, ct, :, o * 128:(o + 1) * 128]
                                                rhs = xn8_v[:, :, base + 4 * ct:base + 4 * ct + 512]
                                                nc.tensor.matmul(
                                                    osl, lhsT, rhs,
                                                    start=(ct == 0), stop=(ct == nct - 1),
                                                    perf_mode=mybir.MatmulPerfMode.DoubleRow,
                                                )
                                            else:
                                                lhsT = wi_v[:, li - 1, d, ct, o * 128:(o + 1) * 128]
                                                rhs = hin[ct][:, base:base + 512]
                                                nc.tensor.matmul(
                                                    osl, lhsT, rhs,
                                                    start=(ct == 0), stop=(ct == nct - 1),
                                                )
                                # evacuate span. d=1 stores each 128-block
                                # reversed (pads land at l' in [0,7)).
                                ssl = slice(s4 * SPAN, (s4 + 1) * SPAN)
                                f_src = fr_ps[:, 0:SPAN].rearrange("p (n l) -> p n l", l=128)
                                r_src = fr_ps[:, SPAN:2 * SPAN].rearrange("p (n l) -> p n l", l=128)
                                zw_src = zw_ps[:].rearrange("p (w n l) -> p w n l", w=2, l=128)
                                if d == 1:
                                    f_src = f_src[:, :, ::-1]
                                    r_src = r_src[:, :, ::-1]
                                    zw_src = zw_src[:, :, :, ::-1]
                                # f-gate weights are negated host-side, so
                                # sigmoid(psum*psc + (-bf)) = 1 - f = g.
                                nc.scalar.activation(f_t[:, ssl], f_src, sig, bias=bcol, scale=psc)
                                nc.scalar.activation(r_t[:, ssl], r_src, sig, bias=rcol, scale=psc)
                                z3 = z_t[:, ssl].rearrange("p (n l) -> p n l", l=128)
                                w3 = w_t[:, ssl].rearrange("p (n l) -> p n l", l=128)
                                if li == 0:
                                    # z|hp scaled back by 1/W0S during evac
                                    nc.scalar.activation(z3, zw_src[:, 0], AF.Copy, scale=psc)
                                    nc.scalar.activation(w3, zw_src[:, 1], AF.Copy, scale=psc)
                                else:
                                    nc.scalar.copy(z3, zw_src[:, 0])
                                    nc.scalar.copy(w3, zw_src[:, 1])
                            # b = g*z (in place over z; read g before the 1-g
                            # pass below overwrites it)
                            nc.vector.tensor_mul(z_h, f_t[:], z_h)
                            # f = 1 - g
                            nc.vector.tensor_scalar(f_t[:], f_t[:], -1.0, 1.0,
                                                    op0=OP.mult, op1=OP.add)
                            # pads reset the scan carry between sequences
                            pads = slice(121, 128) if d == 0 else slice(0, 7)
                            f_v = f_t[:].rearrange("p (n l) -> p n l", l=128)
                            b_v = z_h.rearrange("p (n l) -> p n l", l=128)
                            nc.gpsimd.memset(f_v[:, :, pads], 0.0)
                            nc.gpsimd.memset(b_v[:, :, pads], 0.0)
                            # c = f*c + (1-f)*z
                            nc.vector.tensor_tensor_scan(
                                z_h, f_t[:], z_h, 0.0,
                                op0=OP.mult, op1=OP.add,
                            )
                            # highway: out = r*(cs-hp) + hp
                            nc.vector.tensor_sub(f_t[:], z_h, w_h)
                            nc.vector.tensor_mul(r_t[:], r_t[:], f_t[:])
                            hov = hout[d][:, ooff:ooff + NF].rearrange(
                                "p (n l) -> p n l", l=128
                            )
                            dst = hov[:, half * 32:half * 32 + 32, :]
                            if d == 1:
                                dst = dst[:, :, ::-1]
                            r_v = r_t[:].rearrange("p (n l) -> p n l", l=128)
                            w_v = w_h.rearrange("p (n l) -> p n l", l=128)
                            nc.vector.tensor_add(dst, r_v[:, :, :], w_v[:, :, :])

            # ================= transposed conv + residual =================
            h4 = [h_t[2], h_t[3]]  # layer 3 writes pair B at offset 8
            for t4 in h4:
                v = t4[:, 0:NF].rearrange("p (n l) -> p n l", l=128)
                nc.gpsimd.memset(t4[:, 0:8], 0.0)
                nc.gpsimd.memset(v[:, 1:33, 1:8], 0.0)
                nc.gpsimd.memset(v[:, 33:64, 1:8], 0.0)
                nc.gpsimd.memset(t4[:, NF + 1:XCOLS], 0.0)
            with (
                tc.tile_pool(name="cvp", bufs=4, space="PSUM") as cvp,
                tc.tile_pool(name="osp", bufs=4) as osp,
            ):
                # column-tiled: chunk pair (2j, 2j+1) runs concurrently in PE
                # col-groups 0:64 / 64:128. Reversed: the last pairs need
                # half-1 data (finished first), overlapping layer 3's tail.
                for pair in reversed(range(NF // 1024)):
                    cA = 2 * pair
                    cB = 2 * pair + 1
                    c_ps = cvp.tile([128, 512], F32, tag="c")
                    mm = 0
                    for ct in range(2):
                        for k in range(8):
                            bA = cA * 512 + 8 - k
                            bB = cB * 512 + 8 - k
                            nc.tensor.matmul(
                                c_ps[0:64, :], cw_v[:, ct, k, :], h4[ct][:, bA:bA + 512],
                                start=(mm == 0), stop=(mm == 15), tile_position=(0, 0),
                            )
                            nc.tensor.matmul(
                                c_ps[64:128, :], cw_v[:, ct, k, :], h4[ct][:, bB:bB + 512],
                                start=(mm == 0), stop=(mm == 15), tile_position=(0, 64),
                            )
                            mm += 1
                    o_t = osp.tile([128, 512], BF16, tag="o")
                    slA = slice(cA * 512, cA * 512 + 512)
                    slB = slice(cB * 512, cB * 512 + 512)
                    nc.vector.scalar_tensor_tensor(
                        o_t[0:64, :], c_ps[0:64, :], cb_t[0:64, 0:1], xn2_t[0:64, slA],
                        op0=OP.add, op1=OP.add,
                    )
                    # rows 64:128 read the residual from the shifted xn2 rows
                    nc.vector.scalar_tensor_tensor(
                        o_t[64:128, :], c_ps[64:128, :], cb_t[64:128, 0:1],
                        xn2_t[64:128, cB * 512 - 1:cB * 512 + 511],
                        op0=OP.add, op1=OP.add,
                    )
                    nc.sync.dma_start(out_d[:, slA], o_t[0:64, :])
                    nc.sync.dma_start(out_d[:, slB], o_t[64:128, :])

    nc.compile()
    return nc


def _prep_weights(W0, Ws, convW):
    w0r = W0.reshape(C, K, 2, 4 * H)
    w0p = np.zeros((2, 4, 128, 512), np.float32)
    for d in range(2):
        for kp in range(4):
            w0p[d, kp, 0:64] = w0r[:, 2 * kp, d]
            w0p[d, kp, 64:128] = w0r[:, 2 * kp + 1, d]
    wip = np.zeros((3, 2, 2, 128, 512), np.float32)
    for i in range(3):
        for d in range(2):
            for ct in range(2):
                wip[i, d, ct] = Ws[i][ct * 128:(ct + 1) * 128, d]
    # negate the f-gate output chunk so sigmoid(psum - bf) = 1-f = g
    # without a scale=-1 activation.
    w0p[:, :, :, 128:256] *= -1.0
    wip[:, :, :, :, 128:256] *= -1.0
    # layer-0 weights as fp8 DoubleRow pairs, scaled by W0S to stay out of
    # the e4m3 subnormal range (compensated by psum scale 1/W0S on-chip):
    # w0f8[d, ctp, p, ko, m] = W0S * w0p[d, 2*ctp+ko, p, m]
    w0f8 = np.zeros((2, 2, 128, 2, 512), np.float32)
    for ctp in range(2):
        for ko in range(2):
            w0f8[:, ctp, :, ko, :] = W0S * w0p[:, 2 * ctp + ko].transpose(0, 1, 2)
    cwp = np.zeros((2, 8, 128, C), np.float32)
    for ct in range(2):
        for k in range(8):
            cwp[ct, k] = convW[ct * 128:(ct + 1) * 128, :, k]
    bf16 = ml_dtypes.bfloat16
    f8 = ml_dtypes.float8_e4m3
    return w0f8.astype(f8), wip.astype(bf16), cwp.astype(bf16)


def kernel(**inputs):
    inputs = {k: np.asarray(v) for k, v in inputs.items()}
    x = inputs["x"].astype(np.float32)
    xs = np.ascontiguousarray(
        x.transpose(0, 2, 1, 3).reshape(B * T, C, F_)
    )  # (512, C, F)

    w0f8, wip, cwp = _prep_weights(
        inputs["W0"].astype(np.float32),
        [inputs[f"W{i}"].astype(np.float32) for i in (1, 2, 3)],
        inputs["convW"].astype(np.float32),
    )
    bfp = -np.stack([inputs[f"bf{i}"] for i in range(4)]).astype(np.float32)
    brp = np.stack([inputs[f"br{i}"] for i in range(4)]).astype(np.float32)
    gm = inputs["gamma"].reshape(C).astype(np.float32)
    bt = inputs["beta"].reshape(C).astype(np.float32)
    cb = np.tile(inputs["convb"].reshape(C).astype(np.float32), 2)  # [128]

    if "nc" not in _CACHE:
        _CACHE["nc"] = _build()
    nc = _CACHE["nc"]

    bf16 = ml_dtypes.bfloat16
    shared = {"w0f8": w0f8, "wip": wip, "cwp": cwp, "bfp": bfp, "brp": brp,
              "gm": gm, "bt": bt, "cb": cb}
    in_maps = []
    for core in range(NCORES):
        sh = xs[core * NLOC:(core + 1) * NLOC]  # (NLOC, C, F)
        u = np.ascontiguousarray(sh.transpose(1, 0, 2)).astype(bf16)  # (C, NLOC, F)
        in_maps.append({"u": u, **shared})

    trace = bool(os.environ.get("KBENCH_TRACE"))
    res = bass_utils.run_bass_kernel_spmd(
        nc, in_maps, list(range(NCORES)), trace=trace,
        tmpdir=os.environ.get("KBENCH_TMPDIR"),
    )
    _CACHE["last_result"] = res

    full = np.concatenate(
        [res.results[i]["o"].reshape(C, NLOC, F_) for i in range(NCORES)], axis=1
    )  # (C, 512, F)
    out = full.transpose(1, 0, 2).reshape(B, T, C, F_).transpose(0, 2, 1, 3)
    return np.ascontiguousarray(out.astype(np.float32))


# revision 52
# speedup vs baseline: 1.0588x; 1.0078x over previous
"""Trainium2 Bass kernel for the DPRNN block (channel-norm -> unfold ->
4x bidirectional SRU -> conv-transpose -> residual).

Sharding: data-parallel over the B*T=512 sequences; 64 sequences per core.
All weights replicated. Each core runs the full pipeline on its shard.

Layout (per core): sequences live in 128-column blocks (121 valid SRU steps
+ 7 pad columns). Pads carry f=0, b=0 through the scan so a single
tensor_tensor_scan over the whole free dim handles all sequences.

v3 over v2:
- norm: bf16 input, A/B scale-offset rows broadcast via DMA (no gpsimd
  partition_broadcast, no fp32 xn tile, no scalar-engine chunk copies);
  residual comes from the bf16 xn2 tile.
- SRU: f-gate weights negated host-side so both sigmoids run at
  scale=+1; f/r matmuls issued first so sigmoids start earlier;
  per-(o,ct) matmul ordering halves LDWEIGHTS pressure.
- conv: column-tiled matmuls (two 64-wide output tiles run concurrently
  in the PE array), residual add uses the shifted bf16 xn2 rows.
"""
import os
import numpy as np
import ml_dtypes

import concourse.bass as bass
import concourse.mybir as mybir
import concourse.tile as tile
from concourse import bacc
from concourse import bass_utils

F32 = mybir.dt.float32
BF16 = mybir.dt.bfloat16
FP8 = mybir.dt.float8e4

B, C, T, F_ = 4, 64, 128, 128
H, K = 128, 8
L = F_ - K + 1            # 121
EPS = 1e-8
NCORES = 8
NLOC = (B * T) // NCORES  # 64 sequences per core
NF = NLOC * 128           # 8192
XCOLS = NF + 8            # xn2 / h tiles carry 8 extra cols for shifted reads

DT_H = BF16       # h / xn2 / gate dtype (matmul inputs)
SPAN = 1024       # psum evacuation span (8 seqs)
NSPAN = NF // SPAN
HSPAN = NSPAN // 2
XC8 = NF + 16     # fp8 xn plane stride (16B-aligned for DoubleRow)
W0S = 16.0        # host-side fp8 weight scale for layer 0

_CACHE = {}


def _build():
    nc = bacc.Bacc("TRN2", target_bir_lowering=False, debug=False)
    AF = mybir.ActivationFunctionType
    OP = mybir.AluOpType

    # ---------------- DRAM tensors ----------------
    u_d = nc.dram_tensor("u", [C, NLOC, F_], BF16, kind="ExternalInput").ap()
    w0_d = nc.dram_tensor("w0f8", [2, 2, 128, 2, 512], FP8, kind="ExternalInput").ap()
    wi_d = nc.dram_tensor("wip", [3, 2, 2, 128, 512], BF16, kind="ExternalInput").ap()
    cw_d = nc.dram_tensor("cwp", [2, 8, 128, 64], BF16, kind="ExternalInput").ap()
    bf_d = nc.dram_tensor("bfp", [4, 2, 128], F32, kind="ExternalInput").ap()
    br_d = nc.dram_tensor("brp", [4, 2, 128], F32, kind="ExternalInput").ap()
    gm_d = nc.dram_tensor("gm", [C], F32, kind="ExternalInput").ap()
    bt_d = nc.dram_tensor("bt", [C], F32, kind="ExternalInput").ap()
    cb_d = nc.dram_tensor("cb", [128], F32, kind="ExternalInput").ap()
    out_d = nc.dram_tensor("o", [C, NF], BF16, kind="ExternalOutput").ap()

    with tile.TileContext(nc) as tc:
        with tc.tile_pool(name="const", bufs=1) as cp:
            # ---- weights / biases resident in SBUF ----
            bfp_t = cp.tile([128, 8], F32)
            nc.sync.dma_start(bfp_t[:].rearrange("p (i d) -> p i d", i=4), bf_d.rearrange("i d p -> p i d"))
            brp_t = cp.tile([128, 8], F32)
            nc.sync.dma_start(brp_t[:].rearrange("p (i d) -> p i d", i=4), br_d.rearrange("i d p -> p i d"))
            cb_t = cp.tile([128, 1], F32)   # convb duplicated over both halves
            nc.sync.dma_start(cb_t[:], cb_d.rearrange("(c a) -> c a", a=1))

            # ---- long-lived activations ----
            xn2_t = cp.tile([128, XCOLS], DT_H)   # [xn ; xn shifted by 1] bf16
            h_t = [cp.tile([128, XCOLS], DT_H, name=f"h{i}") for i in range(4)]  # ping-pong pairs
            # fp8 copy of xn2 for the DoubleRow layer-0 matmuls: plane 0 is
            # xn2, plane 1 is xn2 shifted by 2 cols (the second 128-row
            # contraction half, i.e. taps +2/+3 of each weight chunk).
            xn8_t = cp.tile([128, 2 * XC8], FP8)
            xn8_v = xn8_t[:].rearrange("p (e x) -> p e x", e=2)

            nc.gpsimd.memset(xn2_t[:, NF:XCOLS], 0.0)
            nc.gpsimd.memset(xn2_t[64:128, NF - 1:NF], 0.0)
            nc.gpsimd.memset(xn8_v[:, :, NF:XC8], 0.0)

            # tiles only — the weight DMAs are issued after the u DMA below
            # so the stats input gets the bandwidth head start (w0 is needed
            # ~90us in, wi ~150us in).
            w0_t = cp.tile([128, 2 * 2 * 2 * 512], FP8)
            w0_v = w0_t[:].rearrange("p (d cp2 ko m) -> p d cp2 ko m", d=2, cp2=2, ko=2)
            wi_t = cp.tile([128, 3 * 2 * 2 * 512], BF16)
            wi_v = wi_t[:].rearrange("p (i d ct m) -> p i d ct m", i=3, d=2, ct=2)
            cw_t = cp.tile([128, 2 * 8 * 64], BF16)
            cw_v = cw_t[:].rearrange("p (ct k m) -> p ct k m", ct=2, k=8)

            # ================= channel norm =================
            # stats per (n, f) over c via matmul with a [128, 2] ones lhsT:
            # psum row 0 = mean(u), row 1 = mean(u^2). Scale/offset rows
            # A = rsqrt(var+eps), B = -mu*A are broadcast to the 64 channel
            # partitions by rank-1 matmuls with gamma/beta as the lhsT
            # column, folding the affine into the broadcast:
            #   psumA = gamma (x) A ; psumB = gamma (x) B + beta (x) 1.
            with (
                tc.tile_pool(name="normu", bufs=1) as np_,
                tc.tile_pool(name="normst", bufs=2, space="PSUM") as nst,
                tc.tile_pool(name="normbc", bufs=1, space="PSUM") as nbc,
                tc.tile_pool(name="normab", bufs=2) as nab,
            ):
                usq = np_.tile([128, NF], BF16)    # 0:64 u, 64:128 u^2
                # group 1 (seqs 32:64) first: its stats/apply feed L0 half 1
                nc.sync.dma_start(usq[0:64, NF // 2:NF],
                                  u_d[:, NLOC // 2:NLOC, :].rearrange("c n f -> c (n f)"))
                nc.sync.dma_start(usq[0:64, 0:NF // 2],
                                  u_d[:, 0:NLOC // 2, :].rearrange("c n f -> c (n f)"))
                # weight DMAs queue behind u
                nc.scalar.dma_start(w0_v, w0_d.rearrange("d cp2 p ko m -> p d cp2 ko m"))
                nc.scalar.dma_start(cw_v, cw_d.rearrange("ct k p m -> p ct k m"))
                nc.scalar.dma_start(wi_v, wi_d.rearrange("i d ct p m -> p i d ct m"))
                # PE warmup: HAM un-throttles (1.2 -> 2.4 GHz) only after
                # ~3.4us of sustained matmul activity. Burn dummy matmuls on
                # scratch data while the u DMA is in flight (kept short so
                # the stats matmuls aren't stuck behind them in the PE FIFO).
                scr = np_.tile([128, 512], BF16)
                nc.vector.memset(scr[:], 0.0)
                for wi_ in range(24):
                    wm = nst.tile([2, 512], F32, tag="warm")
                    nc.tensor.matmul(wm[:], scr[:, 0:2], scr[:], start=True, stop=True)
                ones2 = np_.tile([128, 2], BF16)
                nc.vector.memset(ones2[:], 0.0)
                nc.vector.memset(ones2[0:64, 0:1], 1.0 / C)
                nc.vector.memset(ones2[64:128, 1:2], 1.0 / C)
                stf = np_.tile([2, 1024], F32)      # stats staging (mu|s2 rows)
                stT = np_.tile([NLOC, 256], F32)    # rows n: cols 0:128 mu, 128:256 s2
                A_t = np_.tile([NLOC, 128], BF16)   # rstd (per seq-row, per f)
                B_t = np_.tile([NLOC, 128], BF16)   # -mu*rstd
                sc1 = np_.tile([NLOC, 128], F32)
                eps_t = np_.tile([NLOC, 1], F32)
                nc.vector.memset(eps_t[:], EPS)
                gmf = np_.tile([1, C], F32)
                nc.sync.dma_start(gmf[:], gm_d.rearrange("(a c) -> a c", a=1))
                btf = np_.tile([1, C], F32)
                nc.sync.dma_start(btf[:], bt_d.rearrange("(a c) -> a c", a=1))
                gmr = np_.tile([1, C], BF16)        # gamma as a lhsT row
                nc.vector.tensor_copy(gmr[:], gmf[:])
                btr = np_.tile([1, C], BF16)        # beta as a lhsT row
                nc.vector.tensor_copy(btr[:], btf[:])
                one5 = np_.tile([1, 512], BF16)
                nc.vector.memset(one5[:], 1.0)

                CH = 1024
                # stats for BOTH groups first: group-0 stats (PE/ACT) then
                # overlap group-1's applies (DVE) instead of queueing after
                # them on the in-order engines.
                for g in (1, 0):
                    gsl = slice(g * (NF // 2), (g + 1) * (NF // 2))
                    nr = slice(g * 32, g * 32 + 32)
                    nc.scalar.activation(usq[64:128, gsl], usq[0:64, gsl],
                                         AF.Square, bias=0.0)
                    for j in range(8):
                        st_ps = nst.tile([2, 512], F32, tag="st")
                        nc.tensor.matmul(st_ps[:], ones2[:],
                                         usq[:, g * 4096 + j * 512:g * 4096 + (j + 1) * 512],
                                         start=True, stop=True)
                        nc.scalar.copy(stf[:, (j % 2) * 512:(j % 2) * 512 + 512], st_ps[:])
                        if j % 2 == 1:
                            # transpose: row 0 (mu) / row 1 (s2) -> [8 rows, 128]
                            nq = slice(g * 32 + (j // 2) * 8, g * 32 + (j // 2) * 8 + 8)
                            nc.sync.dma_start(stT[nq, 0:128], stf[0:1, :])
                            nc.sync.dma_start(stT[nq, 128:256], stf[1:2, :])
                    # dummy matmul chained on the transpose keeps HAM warm
                    wm = nst.tile([2, 512], F32, tag="warm")
                    nc.tensor.matmul(wm[:, 0:256], stT[nr, 0:2], stT[nr, :],
                                     start=True, stop=True)
                    mu_v = stT[nr, 0:128]
                    s2_v = stT[nr, 128:256]
                    nc.vector.tensor_mul(sc1[nr, :], mu_v, mu_v)
                    nc.vector.tensor_sub(s2_v, s2_v, sc1[nr, :])   # var
                    nc.scalar.activation(sc1[nr, :], s2_v, AF.Sqrt, bias=eps_t[nr, 0:1])
                    with nc.allow_low_precision(reason="rstd rounded to bf16 for the bf16 broadcast matmul"):
                        nc.vector.reciprocal(A_t[nr, :], sc1[nr, :])   # rstd
                    nc.vector.scalar_tensor_tensor(
                        B_t[nr, :], mu_v, -1.0, A_t[nr, :], op0=OP.mult, op1=OP.mult
                    )
                    wm = nst.tile([2, 512], F32, tag="warm")
                    nc.tensor.matmul(wm[:, 0:128], stT[nr, 0:2], stT[nr, 0:128],
                                     start=True, stop=True)
                # broadcast A|B chunk rows to the 64 channel partitions as
                # rank-1 matmuls with gamma/beta lhsT columns (folds the
                # affine), then xn = psumA*u + psumB on DVE.
                for ch in range(7, -1, -1):
                    rs = slice(ch * 8, ch * 8 + 8)
                    ab1 = nab.tile([1, 2 * CH], BF16, tag="ab1")
                    nc.sync.dma_start(ab1[:, 0:CH], A_t[rs, :])
                    nc.sync.dma_start(ab1[:, CH:2 * CH], B_t[rs, :])
                    abP = nbc.tile([64, 2 * CH], F32, tag="ab")
                    for s5 in range(2):
                        q = slice(s5 * 512, s5 * 512 + 512)
                        nc.tensor.matmul(abP[:, q], gmr[:], ab1[:, q],
                                         start=True, stop=True)
                        q2 = slice(CH + s5 * 512, CH + s5 * 512 + 512)
                        nc.tensor.matmul(abP[:, q2], gmr[:], ab1[:, q2],
                                         start=True, stop=False)
                        nc.tensor.matmul(abP[:, q2], btr[:], one5[:],
                                         start=False, stop=True)
                    sl = slice(ch * CH, (ch + 1) * CH)
                    nc.vector.tensor_mul(xn2_t[0:64, sl], usq[0:64, sl], abP[:, 0:CH])
                    nc.vector.tensor_add(xn2_t[0:64, sl], xn2_t[0:64, sl], abP[:, CH:2 * CH])
                    # bf16 shifted copy into rows 64:128 (chunk ch reads
                    # chunk ch+1's first col, already written because
                    # chunks go in reverse order)
                    hi = min((ch + 1) * CH + 1, NF)
                    nc.scalar.copy(
                        xn2_t[64:128, ch * CH:hi - 1],
                        xn2_t[0:64, ch * CH + 1:hi],
                    )
                    # fp8 planes for layer 0 (plane 1 reads 2 cols into
                    # the next chunk, already written in reverse order)
                    nc.scalar.copy(xn8_v[:, 0, sl], xn2_t[:, sl])
                    nc.vector.tensor_copy(
                        xn8_v[:, 1, sl],
                        xn2_t[:, ch * CH + 2:(ch + 1) * CH + 2],
                    )

            # ================= SRU layers =================
            sig = AF.Sigmoid
            with (
                tc.tile_pool(name="gates", bufs=2) as gp,
                tc.tile_pool(name="lps", bufs=1, space="PSUM") as pp,
            ):
                for li in range(4):
                    if li == 0:
                        hin = None
                        nct = 2   # two DoubleRow chunks of 256 contraction
                        psc = 1.0 / W0S
                    else:
                        hin = [h_t[2 * ((li - 1) % 2)], h_t[2 * ((li - 1) % 2) + 1]]
                        nct = 2
                        psc = 1.0
                    hout = [h_t[2 * (li % 2)], h_t[2 * (li % 2) + 1]]
                    ooff = 8 if li == 3 else 0
                    for half in (1, 0):
                        for d in range(2):
                            bcol = bfp_t[:, 2 * li + d:2 * li + d + 1]
                            rcol = brp_t[:, 2 * li + d:2 * li + d + 1]
                            f_t = gp.tile([128, NF // 2], DT_H, tag="f")  # g, then 1-g, then c-hp
                            r_t = gp.tile([128, NF // 2], DT_H, tag="r")
                            z_t = gp.tile([128, NF // 2], DT_H, tag="z")  # z, then b
                            w_t = gp.tile([128, NF // 2], DT_H, tag="w")  # hp
                            c_t = gp.tile([128, NF // 2], DT_H, tag="c")  # scan output
                            z_h = z_t[:]
                            w_h = w_t[:]
                            for s4 in range(HSPAN):
                                span = half * HSPAN + s4
                                fr_ps = pp.tile([128, 2 * SPAN], F32, name="fr", tag="fr")
                                zw_ps = pp.tile([128, 2 * SPAN], F32, name="zw", tag="zw")
                                # f/r matmuls first so the sigmoids start
                                # early; per-(o,ct) inner pairing reuses each
                                # weight for two 512-col matmuls.
                                pst = [(1, fr_ps[:, 0:SPAN]), (2, fr_ps[:, SPAN:2 * SPAN]),
                                       (0, zw_ps[:, 0:SPAN]), (3, zw_ps[:, SPAN:2 * SPAN])]
                                for o, dst in pst:
                                    for ct in range(nct):
                                        for h2 in range(SPAN // 512):
                                            osl = dst[:, h2 * 512:(h2 + 1) * 512]
                                            base = span * SPAN + h2 * 512
                                            if li == 0:
                                                lhsT = w0_v[:, d, ct, :, o * 128:(o + 1) * 128]
                                                rhs = xn8_v[:, :, base + 4 * ct:base + 4 * ct + 512]
                                                nc.tensor.matmul(
                                                    osl, lhsT, rhs,
                                                    start=(ct == 0), stop=(ct == nct - 1),
                                                    perf_mode=mybir.MatmulPerfMode.DoubleRow,
                                                )
                                            else:
                                                lhsT = wi_v[:, li - 1, d, ct, o * 128:(o + 1) * 128]
                                                rhs = hin[ct][:, base:base + 512]
                                                nc.tensor.matmul(
                                                    osl, lhsT, rhs,
                                                    start=(ct == 0), stop=(ct == nct - 1),
                                                )
                                # evacuate span. d=1 stores each 128-block
                                # reversed (pads land at l' in [0,7)).
                                ssl = slice(s4 * SPAN, (s4 + 1) * SPAN)
                                f_src = fr_ps[:, 0:SPAN].rearrange("p (n l) -> p n l", l=128)
                                r_src = fr_ps[:, SPAN:2 * SPAN].rearrange("p (n l) -> p n l", l=128)
                                zw_src = zw_ps[:].rearrange("p (w n l) -> p w n l", w=2, l=128)
                                if d == 1:
                                    f_src = f_src[:, :, ::-1]
                                    r_src = r_src[:, :, ::-1]
                                    zw_src = zw_src[:, :, :, ::-1]
                                # f-gate weights are negated host-side, so
                                # sigmoid(psum*psc + (-bf)) = 1 - f = g.
                                nc.scalar.activation(f_t[:, ssl], f_src, sig, bias=bcol, scale=psc)
                                nc.scalar.activation(r_t[:, ssl], r_src, sig, bias=rcol, scale=psc)
                                z3 = z_t[:, ssl].rearrange("p (n l) -> p n l", l=128)
                                w3 = w_t[:, ssl].rearrange("p (n l) -> p n l", l=128)
                                if li == 0:
                                    # z|hp scaled back by 1/W0S during evac
                                    nc.scalar.activation(z3, zw_src[:, 0], AF.Copy, scale=psc)
                                    nc.scalar.activation(w3, zw_src[:, 1], AF.Copy, scale=psc)
                                else:
                                    nc.scalar.copy(z3, zw_src[:, 0])
                                    nc.scalar.copy(w3, zw_src[:, 1])
                            # b = g*z (in place over z; read g before the 1-g
                            # pass below overwrites it)
                            nc.vector.tensor_mul(z_h, f_t[:], z_h)
                            # f = 1 - g
                            nc.vector.tensor_scalar(f_t[:], f_t[:], -1.0, 1.0,
                                                    op0=OP.mult, op1=OP.add)
                            # pads reset the scan carry between sequences
                            pads = slice(121, 128) if d == 0 else slice(0, 7)
                            f_v = f_t[:].rearrange("p (n l) -> p n l", l=128)
                            b_v = z_h.rearrange("p (n l) -> p n l", l=128)
                            nc.gpsimd.memset(f_v[:, :, pads], 0.0)
                            nc.gpsimd.memset(b_v[:, :, pads], 0.0)
                            # c = f*c + (1-f)*z  (separate output tile: an
                            # in-place scan serializes at 2 cyc/elem)
                            nc.vector.tensor_tensor_scan(
                                c_t[:], f_t[:], z_h, 0.0,
                                op0=OP.mult, op1=OP.add,
                            )
                            # highway: out = r*(cs-hp) + hp
                            nc.vector.tensor_sub(f_t[:], c_t[:], w_h)
                            nc.vector.tensor_mul(r_t[:], r_t[:], f_t[:])
                            hov = hout[d][:, ooff:ooff + NF].rearrange(
                                "p (n l) -> p n l", l=128
                            )
                            dst = hov[:, half * 32:half * 32 + 32, :]
                            if d == 1:
                                dst = dst[:, :, ::-1]
                            r_v = r_t[:].rearrange("p (n l) -> p n l", l=128)
                            w_v = w_h.rearrange("p (n l) -> p n l", l=128)
                            nc.vector.tensor_add(dst, r_v[:, :, :], w_v[:, :, :])

            # ================= transposed conv + residual =================
            h4 = [h_t[2], h_t[3]]  # layer 3 writes pair B at offset 8
            for t4 in h4:
                v = t4[:, 0:NF].rearrange("p (n l) -> p n l", l=128)
                nc.gpsimd.memset(t4[:, 0:8], 0.0)
                nc.gpsimd.memset(v[:, 1:33, 1:8], 0.0)
                nc.gpsimd.memset(v[:, 33:64, 1:8], 0.0)
                nc.gpsimd.memset(t4[:, NF + 1:XCOLS], 0.0)
            with (
                tc.tile_pool(name="cvp", bufs=4, space="PSUM") as cvp,
                tc.tile_pool(name="osp", bufs=4) as osp,
            ):
                # column-tiled: chunk pair (2j, 2j+1) runs concurrently in PE
                # col-groups 0:64 / 64:128. Reversed: the last pairs need
                # half-1 data (finished first), overlapping layer 3's tail.
                for pair in reversed(range(NF // 1024)):
                    cA = 2 * pair
                    cB = 2 * pair + 1
                    c_ps = cvp.tile([128, 512], F32, tag="c")
                    mm = 0
                    for ct in range(2):
                        for k in range(8):
                            bA = cA * 512 + 8 - k
                            bB = cB * 512 + 8 - k
                            nc.tensor.matmul(
                                c_ps[0:64, :], cw_v[:, ct, k, :], h4[ct][:, bA:bA + 512],
                                start=(mm == 0), stop=(mm == 15), tile_position=(0, 0),
                            )
                            nc.tensor.matmul(
                                c_ps[64:128, :], cw_v[:, ct, k, :], h4[ct][:, bB:bB + 512],
                                start=(mm == 0), stop=(mm == 15), tile_position=(0, 64),
                            )
                            mm += 1
                    o_t = osp.tile([128, 512], BF16, tag="o")
                    slA = slice(cA * 512, cA * 512 + 512)
                    slB = slice(cB * 512, cB * 512 + 512)
                    nc.vector.scalar_tensor_tensor(
                        o_t[0:64, :], c_ps[0:64, :], cb_t[0:64, 0:1], xn2_t[0:64, slA],
                        op0=OP.add, op1=OP.add,
                    )
                    # rows 64:128 read the residual from the shifted xn2 rows
                    nc.vector.scalar_tensor_tensor(
                        o_t[64:128, :], c_ps[64:128, :], cb_t[64:128, 0:1],
                        xn2_t[64:128, cB * 512 - 1:cB * 512 + 511],
                        op0=OP.add, op1=OP.add,
                    )
                    nc.sync.dma_start(out_d[:, slA], o_t[0:64, :])
                    nc.sync.dma_start(out_d[:, slB], o_t[64:128, :])

    nc.compile()
    return nc


def _prep_weights(W0, Ws, convW):
    w0r = W0.reshape(C, K, 2, 4 * H)
    w0p = np.zeros((2, 4, 128, 512), np.float32)
    for d in range(2):
        for kp in range(4):
            w0p[d, kp, 0:64] = w0r[:, 2 * kp, d]
            w0p[d, kp, 64:128] = w0r[:, 2 * kp + 1, d]
    wip = np.zeros((3, 2, 2, 128, 512), np.float32)
    for i in range(3):
        for d in range(2):
            for ct in range(2):
                wip[i, d, ct] = Ws[i][ct * 128:(ct + 1) * 128, d]
    # negate the f-gate output chunk so sigmoid(psum - bf) = 1-f = g
    # without a scale=-1 activation.
    w0p[:, :, :, 128:256] *= -1.0
    wip[:, :, :, :, 128:256] *= -1.0
    # layer-0 weights as fp8 DoubleRow pairs, scaled by W0S to stay out of
    # the e4m3 subnormal range (compensated by psum scale 1/W0S on-chip):
    # w0f8[d, ctp, p, ko, m] = W0S * w0p[d, 2*ctp+ko, p, m]
    w0f8 = np.zeros((2, 2, 128, 2, 512), np.float32)
    for ctp in range(2):
        for ko in range(2):
            w0f8[:, ctp, :, ko, :] = W0S * w0p[:, 2 * ctp + ko].transpose(0, 1, 2)
    cwp = np.zeros((2, 8, 128, C), np.float32)
    for ct in range(2):
        for k in range(8):
            cwp[ct, k] = convW[ct * 128:(ct + 1) * 128, :, k]
    bf16 = ml_dtypes.bfloat16
    f8 = ml_dtypes.float8_e4m3
    return w0f8.astype(f8), wip.astype(bf16), cwp.astype(bf16)


def kernel(**inputs):
    inputs = {k: np.asarray(v) for k, v in inputs.items()}
    x = inputs["x"].astype(np.float32)
    xs = np.ascontiguousarray(
        x.transpose(0, 2, 1, 3).reshape(B * T, C, F_)
    )  # (512, C, F)

    w0f8, wip, cwp = _prep_weights(
        inputs["W0"].astype(np.float32),
        [inputs[f"W{i}"].astype(np.float32) for i in (1, 2, 3)],
        inputs["convW"].astype(np.float32),
    )
    bfp = -np.stack([inputs[f"bf{i}"] for i in range(4)]).astype(np.float32)
    brp = np.stack([inputs[f"br{i}"] for i in range(4)]).astype(np.float32)
    gm = inputs["gamma"].reshape(C).astype(np.float32)
    bt = inputs["beta"].reshape(C).astype(np.float32)
    cb = np.tile(inputs["convb"].reshape(C).astype(np.float32), 2)  # [128]

    if "nc" not in _CACHE:
        _CACHE["nc"] = _build()
    nc = _CACHE["nc"]

    bf16 = ml_dtypes.bfloat16
    shared = {"w0f8": w0f8, "wip": wip, "cwp": cwp, "bfp": bfp, "brp": brp,
              "gm": gm, "bt": bt, "cb": cb}
    in_maps = []
    for core in range(NCORES):
        sh = xs[core * NLOC:(core + 1) * NLOC]  # (NLOC, C, F)
        u = np.ascontiguousarray(sh.transpose(1, 0, 2)).astype(bf16)  # (C, NLOC, F)
        in_maps.append({"u": u, **shared})

    trace = bool(os.environ.get("KBENCH_TRACE"))
    res = bass_utils.run_bass_kernel_spmd(
        nc, in_maps, list(range(NCORES)), trace=trace,
        tmpdir=os.environ.get("KBENCH_TMPDIR"),
    )
    _CACHE["last_result"] = res

    full = np.concatenate(
        [res.results[i]["o"].reshape(C, NLOC, F_) for i in range(NCORES)], axis=1
    )  # (C, 512, F)
    out = full.transpose(1, 0, 2).reshape(B, T, C, F_).transpose(0, 2, 1, 3)
    return np.ascontiguousarray(out.astype(np.float32))


# revision 53
# speedup vs baseline: 1.0610x; 1.0021x over previous
"""Trainium2 Bass kernel for the DPRNN block (channel-norm -> unfold ->
4x bidirectional SRU -> conv-transpose -> residual).

Sharding: data-parallel over the B*T=512 sequences; 64 sequences per core.
All weights replicated. Each core runs the full pipeline on its shard.

Layout (per core): sequences live in 128-column blocks (121 valid SRU steps
+ 7 pad columns). Pads carry f=0, b=0 through the scan so a single
tensor_tensor_scan over the whole free dim handles all sequences.

v3 over v2:
- norm: bf16 input, A/B scale-offset rows broadcast via DMA (no gpsimd
  partition_broadcast, no fp32 xn tile, no scalar-engine chunk copies);
  residual comes from the bf16 xn2 tile.
- SRU: f-gate weights negated host-side so both sigmoids run at
  scale=+1; f/r matmuls issued first so sigmoids start earlier;
  per-(o,ct) matmul ordering halves LDWEIGHTS pressure.
- conv: column-tiled matmuls (two 64-wide output tiles run concurrently
  in the PE array), residual add uses the shifted bf16 xn2 rows.
"""
import os
import numpy as np
import ml_dtypes

import concourse.bass as bass
import concourse.mybir as mybir
import concourse.tile as tile
from concourse import bacc
from concourse import bass_utils

F32 = mybir.dt.float32
BF16 = mybir.dt.bfloat16
FP8 = mybir.dt.float8e4

B, C, T, F_ = 4, 64, 128, 128
H, K = 128, 8
L = F_ - K + 1            # 121
EPS = 1e-8
NCORES = 8
NLOC = (B * T) // NCORES  # 64 sequences per core
NF = NLOC * 128           # 8192
XCOLS = NF + 8            # xn2 / h tiles carry 8 extra cols for shifted reads

DT_H = BF16       # h / xn2 / gate dtype (matmul inputs)
SPAN = 1024       # psum evacuation span (8 seqs)
NSPAN = NF // SPAN
HSPAN = NSPAN // 2
XC8 = NF + 16     # fp8 xn plane stride (16B-aligned for DoubleRow)
W0S = 16.0        # host-side fp8 weight scale for layer 0

_CACHE = {}


def _build():
    nc = bacc.Bacc("TRN2", target_bir_lowering=False, debug=False)
    AF = mybir.ActivationFunctionType
    OP = mybir.AluOpType

    # ---------------- DRAM tensors ----------------
    u_d = nc.dram_tensor("u", [C, NLOC, F_], BF16, kind="ExternalInput").ap()
    w0_d = nc.dram_tensor("w0f8", [2, 2, 128, 2, 512], FP8, kind="ExternalInput").ap()
    wi_d = nc.dram_tensor("wip", [3, 2, 2, 128, 512], BF16, kind="ExternalInput").ap()
    cw_d = nc.dram_tensor("cwp", [2, 8, 128, 64], BF16, kind="ExternalInput").ap()
    bf_d = nc.dram_tensor("bfp", [4, 2, 128], F32, kind="ExternalInput").ap()
    br_d = nc.dram_tensor("brp", [4, 2, 128], F32, kind="ExternalInput").ap()
    gm_d = nc.dram_tensor("gm", [C], F32, kind="ExternalInput").ap()
    bt_d = nc.dram_tensor("bt", [C], F32, kind="ExternalInput").ap()
    cb_d = nc.dram_tensor("cb", [128], F32, kind="ExternalInput").ap()
    out_d = nc.dram_tensor("o", [C, NF], BF16, kind="ExternalOutput").ap()

    with tile.TileContext(nc) as tc:
        with tc.tile_pool(name="const", bufs=1) as cp:
            # ---- weights / biases resident in SBUF ----
            bfp_t = cp.tile([128, 8], F32)
            nc.sync.dma_start(bfp_t[:].rearrange("p (i d) -> p i d", i=4), bf_d.rearrange("i d p -> p i d"))
            brp_t = cp.tile([128, 8], F32)
            nc.sync.dma_start(brp_t[:].rearrange("p (i d) -> p i d", i=4), br_d.rearrange("i d p -> p i d"))
            cb_t = cp.tile([128, 1], F32)   # convb duplicated over both halves
            nc.sync.dma_start(cb_t[:], cb_d.rearrange("(c a) -> c a", a=1))

            # ---- long-lived activations ----
            xn2_t = cp.tile([128, XCOLS], DT_H)   # [xn ; xn shifted by 1] bf16
            h_t = [cp.tile([128, XCOLS], DT_H, name=f"h{i}") for i in range(4)]  # ping-pong pairs
            # fp8 copy of xn2 for the DoubleRow layer-0 matmuls: plane 0 is
            # xn2, plane 1 is xn2 shifted by 2 cols (the second 128-row
            # contraction half, i.e. taps +2/+3 of each weight chunk).
            xn8_t = cp.tile([128, 2 * XC8], FP8)
            xn8_v = xn8_t[:].rearrange("p (e x) -> p e x", e=2)

            nc.gpsimd.memset(xn2_t[:, NF:XCOLS], 0.0)
            nc.gpsimd.memset(xn2_t[64:128, NF - 1:NF], 0.0)
            nc.gpsimd.memset(xn8_v[:, :, NF:XC8], 0.0)

            # tiles only — the weight DMAs are issued after the u DMA below
            # so the stats input gets the bandwidth head start (w0 is needed
            # ~90us in, wi ~150us in).
            w0_t = cp.tile([128, 2 * 2 * 2 * 512], FP8)
            w0_v = w0_t[:].rearrange("p (d cp2 ko m) -> p d cp2 ko m", d=2, cp2=2, ko=2)
            wi_t = cp.tile([128, 3 * 2 * 2 * 512], BF16)
            wi_v = wi_t[:].rearrange("p (i d ct m) -> p i d ct m", i=3, d=2, ct=2)
            cw_t = cp.tile([128, 2 * 8 * 64], BF16)
            cw_v = cw_t[:].rearrange("p (ct k m) -> p ct k m", ct=2, k=8)

            # ================= channel norm =================
            # stats per (n, f) over c via matmul with a [128, 2] ones lhsT:
            # psum row 0 = mean(u), row 1 = mean(u^2). Scale/offset rows
            # A = rsqrt(var+eps), B = -mu*A are broadcast to the 64 channel
            # partitions by rank-1 matmuls with gamma/beta as the lhsT
            # column, folding the affine into the broadcast:
            #   psumA = gamma (x) A ; psumB = gamma (x) B + beta (x) 1.
            with (
                tc.tile_pool(name="normu", bufs=1) as np_,
                tc.tile_pool(name="normst", bufs=2, space="PSUM") as nst,
                tc.tile_pool(name="normbc", bufs=1, space="PSUM") as nbc,
                tc.tile_pool(name="normab", bufs=2) as nab,
            ):
                usq = np_.tile([128, NF], BF16)    # 0:64 u, 64:128 u^2
                # group 1 (seqs 32:64) first: its stats/apply feed L0 half 1
                nc.sync.dma_start(usq[0:64, NF // 2:NF],
                                  u_d[:, NLOC // 2:NLOC, :].rearrange("c n f -> c (n f)"))
                nc.sync.dma_start(usq[0:64, 0:NF // 2],
                                  u_d[:, 0:NLOC // 2, :].rearrange("c n f -> c (n f)"))
                # weight DMAs queue behind u
                nc.scalar.dma_start(w0_v, w0_d.rearrange("d cp2 p ko m -> p d cp2 ko m"))
                nc.scalar.dma_start(cw_v, cw_d.rearrange("ct k p m -> p ct k m"))
                nc.scalar.dma_start(wi_v, wi_d.rearrange("i d ct p m -> p i d ct m"))
                # PE warmup: HAM un-throttles (1.2 -> 2.4 GHz) only after
                # ~3.4us of sustained matmul activity. Burn dummy matmuls on
                # scratch data while the u DMA is in flight (kept short so
                # the stats matmuls aren't stuck behind them in the PE FIFO).
                scr = np_.tile([128, 512], BF16)
                nc.vector.memset(scr[:], 0.0)
                for wi_ in range(24):
                    wm = nst.tile([2, 512], F32, tag="warm")
                    nc.tensor.matmul(wm[:], scr[:, 0:2], scr[:], start=True, stop=True)
                ones2 = np_.tile([128, 2], BF16)
                nc.vector.memset(ones2[:], 0.0)
                nc.vector.memset(ones2[0:64, 0:1], 1.0 / C)
                nc.vector.memset(ones2[64:128, 1:2], 1.0 / C)
                stf = np_.tile([2, 1024], F32)      # stats staging (mu|s2 rows)
                stT = np_.tile([NLOC, 256], F32)    # rows n: cols 0:128 mu, 128:256 s2
                A_t = np_.tile([NLOC, 128], BF16)   # rstd (per seq-row, per f)
                B_t = np_.tile([NLOC, 128], BF16)   # -mu*rstd
                sc1 = np_.tile([NLOC, 128], F32)
                eps_t = np_.tile([NLOC, 1], F32)
                nc.vector.memset(eps_t[:], EPS)
                gmf = np_.tile([1, C], F32)
                nc.sync.dma_start(gmf[:], gm_d.rearrange("(a c) -> a c", a=1))
                btf = np_.tile([1, C], F32)
                nc.sync.dma_start(btf[:], bt_d.rearrange("(a c) -> a c", a=1))
                gmr = np_.tile([1, C], BF16)        # gamma as a lhsT row
                nc.vector.tensor_copy(gmr[:], gmf[:])
                btr = np_.tile([1, C], BF16)        # beta as a lhsT row
                nc.vector.tensor_copy(btr[:], btf[:])
                one5 = np_.tile([1, 512], BF16)
                nc.vector.memset(one5[:], 1.0)

                CH = 1024
                # stats for BOTH groups first: group-0 stats (PE/ACT) then
                # overlap group-1's applies (DVE) instead of queueing after
                # them on the in-order engines.
                for g in (1, 0):
                    gsl = slice(g * (NF // 2), (g + 1) * (NF // 2))
                    nr = slice(g * 32, g * 32 + 32)
                    nc.scalar.activation(usq[64:128, gsl], usq[0:64, gsl],
                                         AF.Square, bias=0.0)
                    for j in range(8):
                        st_ps = nst.tile([2, 512], F32, tag="st")
                        nc.tensor.matmul(st_ps[:], ones2[:],
                                         usq[:, g * 4096 + j * 512:g * 4096 + (j + 1) * 512],
                                         start=True, stop=True)
                        nc.scalar.copy(stf[:, (j % 2) * 512:(j % 2) * 512 + 512], st_ps[:])
                        if j % 2 == 1:
                            # transpose: row 0 (mu) / row 1 (s2) -> [8 rows, 128]
                            nq = slice(g * 32 + (j // 2) * 8, g * 32 + (j // 2) * 8 + 8)
                            nc.sync.dma_start(stT[nq, 0:128], stf[0:1, :])
                            nc.sync.dma_start(stT[nq, 128:256], stf[1:2, :])
                    # dummy matmul chained on the transpose keeps HAM warm
                    wm = nst.tile([2, 512], F32, tag="warm")
                    nc.tensor.matmul(wm[:, 0:256], stT[nr, 0:2], stT[nr, :],
                                     start=True, stop=True)
                    mu_v = stT[nr, 0:128]
                    s2_v = stT[nr, 128:256]
                    nc.vector.tensor_mul(sc1[nr, :], mu_v, mu_v)
                    nc.vector.tensor_sub(s2_v, s2_v, sc1[nr, :])   # var
                    nc.scalar.activation(sc1[nr, :], s2_v, AF.Sqrt, bias=eps_t[nr, 0:1])
                    with nc.allow_low_precision(reason="rstd rounded to bf16 for the bf16 broadcast matmul"):
                        nc.vector.reciprocal(A_t[nr, :], sc1[nr, :])   # rstd
                    nc.vector.scalar_tensor_tensor(
                        B_t[nr, :], mu_v, -1.0, A_t[nr, :], op0=OP.mult, op1=OP.mult
                    )
                    wm = nst.tile([2, 512], F32, tag="warm")
                    nc.tensor.matmul(wm[:, 0:128], stT[nr, 0:2], stT[nr, 0:128],
                                     start=True, stop=True)
                # broadcast A|B chunk rows to the 64 channel partitions as
                # rank-1 matmuls with gamma/beta lhsT columns (folds the
                # affine), then xn = psumA*u + psumB on DVE.
                for ch in range(7, -1, -1):
                    rs = slice(ch * 8, ch * 8 + 8)
                    ab1 = nab.tile([1, 2 * CH], BF16, tag="ab1")
                    nc.sync.dma_start(ab1[:, 0:CH], A_t[rs, :])
                    nc.sync.dma_start(ab1[:, CH:2 * CH], B_t[rs, :])
                    abP = nbc.tile([64, 2 * CH], F32, tag="ab")
                    for s5 in range(2):
                        q = slice(s5 * 512, s5 * 512 + 512)
                        nc.tensor.matmul(abP[:, q], gmr[:], ab1[:, q],
                                         start=True, stop=True)
                        q2 = slice(CH + s5 * 512, CH + s5 * 512 + 512)
                        nc.tensor.matmul(abP[:, q2], gmr[:], ab1[:, q2],
                                         start=True, stop=False)
                        nc.tensor.matmul(abP[:, q2], btr[:], one5[:],
                                         start=False, stop=True)
                    sl = slice(ch * CH, (ch + 1) * CH)
                    nc.vector.tensor_mul(xn2_t[0:64, sl], usq[0:64, sl], abP[:, 0:CH])
                    nc.vector.tensor_add(xn2_t[0:64, sl], xn2_t[0:64, sl], abP[:, CH:2 * CH])
                    # bf16 shifted copy into rows 64:128 (chunk ch reads
                    # chunk ch+1's first col, already written because
                    # chunks go in reverse order)
                    hi = min((ch + 1) * CH + 1, NF)
                    nc.scalar.copy(
                        xn2_t[64:128, ch * CH:hi - 1],
                        xn2_t[0:64, ch * CH + 1:hi],
                    )
                    # fp8 planes for layer 0 (plane 1 reads 2 cols into
                    # the next chunk, already written in reverse order)
                    nc.scalar.copy(xn8_v[:, 0, sl], xn2_t[:, sl])
                    nc.vector.tensor_copy(
                        xn8_v[:, 1, sl],
                        xn2_t[:, ch * CH + 2:(ch + 1) * CH + 2],
                    )

            # ================= SRU layers =================
            sig = AF.Sigmoid
            with (
                tc.tile_pool(name="gates", bufs=2) as gp,
                tc.tile_pool(name="lps", bufs=1, space="PSUM") as pp,
            ):
                for li in range(4):
                    if li == 0:
                        hin = None
                        nct = 2   # two DoubleRow chunks of 256 contraction
                        psc = 1.0 / W0S
                    else:
                        hin = [h_t[2 * ((li - 1) % 2)], h_t[2 * ((li - 1) % 2) + 1]]
                        nct = 2
                        psc = 1.0
                    hout = [h_t[2 * (li % 2)], h_t[2 * (li % 2) + 1]]
                    ooff = 8 if li == 3 else 0
                    for half in (1, 0):
                        for d in range(2):
                            bcol = bfp_t[:, 2 * li + d:2 * li + d + 1]
                            rcol = brp_t[:, 2 * li + d:2 * li + d + 1]
                            f_t = gp.tile([128, NF // 2], DT_H, tag="f")  # g, then 1-g, then c-hp
                            r_t = gp.tile([128, NF // 2], DT_H, tag="r")
                            z_t = gp.tile([128, NF // 2], DT_H, tag="z")  # z, then b
                            w_t = gp.tile([128, NF // 2], DT_H, tag="w")  # hp
                            c_t = gp.tile([128, NF // 2], DT_H, tag="c")  # scan output
                            z_h = z_t[:]
                            w_h = w_t[:]
                            for s4 in range(HSPAN):
                                span = half * HSPAN + s4
                                fr_ps = pp.tile([128, 2 * SPAN], F32, name="fr", tag="fr")
                                zw_ps = pp.tile([128, 2 * SPAN], F32, name="zw", tag="zw")
                                # f/r matmuls first so the sigmoids start
                                # early; per-(o,ct) inner pairing reuses each
                                # weight for two 512-col matmuls.
                                pst = [(1, fr_ps[:, 0:SPAN]), (2, fr_ps[:, SPAN:2 * SPAN]),
                                       (0, zw_ps[:, 0:SPAN]), (3, zw_ps[:, SPAN:2 * SPAN])]
                                for o, dst in pst:
                                    for ct in range(nct):
                                        for h2 in range(SPAN // 512):
                                            osl = dst[:, h2 * 512:(h2 + 1) * 512]
                                            base = span * SPAN + h2 * 512
                                            if li == 0:
                                                lhsT = w0_v[:, d, ct, :, o * 128:(o + 1) * 128]
                                                rhs = xn8_v[:, :, base + 4 * ct:base + 4 * ct + 512]
                                                nc.tensor.matmul(
                                                    osl, lhsT, rhs,
                                                    start=(ct == 0), stop=(ct == nct - 1),
                                                    perf_mode=mybir.MatmulPerfMode.DoubleRow,
                                                )
                                            else:
                                                lhsT = wi_v[:, li - 1, d, ct, o * 128:(o + 1) * 128]
                                                rhs = hin[ct][:, base:base + 512]
                                                nc.tensor.matmul(
                                                    osl, lhsT, rhs,
                                                    start=(ct == 0), stop=(ct == nct - 1),
                                                )
                                # evacuate span. d=1 stores each 128-block
                                # reversed (pads land at l' in [0,7)).
                                ssl = slice(s4 * SPAN, (s4 + 1) * SPAN)
                                f_src = fr_ps[:, 0:SPAN].rearrange("p (n l) -> p n l", l=128)
                                r_src = fr_ps[:, SPAN:2 * SPAN].rearrange("p (n l) -> p n l", l=128)
                                zw_src = zw_ps[:].rearrange("p (w n l) -> p w n l", w=2, l=128)
                                if d == 1:
                                    f_src = f_src[:, :, ::-1]
                                    r_src = r_src[:, :, ::-1]
                                    zw_src = zw_src[:, :, :, ::-1]
                                # f-gate weights are negated host-side, so
                                # sigmoid(psum*psc + (-bf)) = 1 - f = g.
                                nc.scalar.activation(f_t[:, ssl], f_src, sig, bias=bcol, scale=psc)
                                nc.scalar.activation(r_t[:, ssl], r_src, sig, bias=rcol, scale=psc)
                                z3 = z_t[:, ssl].rearrange("p (n l) -> p n l", l=128)
                                w3 = w_t[:, ssl].rearrange("p (n l) -> p n l", l=128)
                                if li == 0:
                                    # z|hp scaled back by 1/W0S during evac
                                    nc.scalar.activation(z3, zw_src[:, 0], AF.Copy, scale=psc)
                                    nc.scalar.activation(w3, zw_src[:, 1], AF.Copy, scale=psc)
                                else:
                                    nc.scalar.copy(z3, zw_src[:, 0])
                                    nc.scalar.copy(w3, zw_src[:, 1])
                            # b = g*z (in place over z; read g before the 1-g
                            # pass below overwrites it)
                            nc.vector.tensor_mul(z_h, f_t[:], z_h)
                            # f = 1 - g
                            nc.vector.tensor_scalar(f_t[:], f_t[:], -1.0, 1.0,
                                                    op0=OP.mult, op1=OP.add)
                            # pads reset the scan carry between sequences
                            pads = slice(121, 128) if d == 0 else slice(0, 7)
                            f_v = f_t[:].rearrange("p (n l) -> p n l", l=128)
                            b_v = z_h.rearrange("p (n l) -> p n l", l=128)
                            nc.gpsimd.memset(f_v[:, :, pads], 0.0)
                            nc.gpsimd.memset(b_v[:, :, pads], 0.0)
                            # c = f*c + (1-f)*z  (separate output tile: an
                            # in-place scan serializes at 2 cyc/elem)
                            nc.vector.tensor_tensor_scan(
                                c_t[:], f_t[:], z_h, 0.0,
                                op0=OP.mult, op1=OP.add,
                            )
                            # highway: out = r*(cs-hp) + hp  (the multiply
                            # writes the dead z tile: in-place DVE ops pay a
                            # read-write aliasing penalty)
                            nc.vector.tensor_sub(f_t[:], c_t[:], w_h)
                            nc.vector.tensor_mul(z_t[:], r_t[:], f_t[:])
                            hov = hout[d][:, ooff:ooff + NF].rearrange(
                                "p (n l) -> p n l", l=128
                            )
                            dst = hov[:, half * 32:half * 32 + 32, :]
                            if d == 1:
                                dst = dst[:, :, ::-1]
                            r_v = z_t[:].rearrange("p (n l) -> p n l", l=128)
                            w_v = w_h.rearrange("p (n l) -> p n l", l=128)
                            nc.vector.tensor_add(dst, r_v[:, :, :], w_v[:, :, :])

            # ================= transposed conv + residual =================
            h4 = [h_t[2], h_t[3]]  # layer 3 writes pair B at offset 8
            for t4 in h4:
                v = t4[:, 0:NF].rearrange("p (n l) -> p n l", l=128)
                nc.gpsimd.memset(t4[:, 0:8], 0.0)
                nc.gpsimd.memset(v[:, 1:33, 1:8], 0.0)
                nc.gpsimd.memset(v[:, 33:64, 1:8], 0.0)
                nc.gpsimd.memset(t4[:, NF + 1:XCOLS], 0.0)
            with (
                tc.tile_pool(name="cvp", bufs=4, space="PSUM") as cvp,
                tc.tile_pool(name="osp", bufs=4) as osp,
            ):
                # column-tiled: chunk pair (2j, 2j+1) runs concurrently in PE
                # col-groups 0:64 / 64:128. Reversed: the last pairs need
                # half-1 data (finished first), overlapping layer 3's tail.
                for pair in reversed(range(NF // 1024)):
                    cA = 2 * pair
                    cB = 2 * pair + 1
                    c_ps = cvp.tile([128, 512], F32, tag="c")
                    mm = 0
                    for ct in range(2):
                        for k in range(8):
                            bA = cA * 512 + 8 - k
                            bB = cB * 512 + 8 - k
                            nc.tensor.matmul(
                                c_ps[0:64, :], cw_v[:, ct, k, :], h4[ct][:, bA:bA + 512],
                                start=(mm == 0), stop=(mm == 15), tile_position=(0, 0),
                            )
                            nc.tensor.matmul(
                                c_ps[64:128, :], cw_v[:, ct, k, :], h4[ct][:, bB:bB + 512],
                                start=(mm == 0), stop=(mm == 15), tile_position=(0, 64),
                            )
                            mm += 1
                    o_t = osp.tile([128, 512], BF16, tag="o")
                    slA = slice(cA * 512, cA * 512 + 512)
                    slB = slice(cB * 512, cB * 512 + 512)
                    nc.vector.scalar_tensor_tensor(
                        o_t[0:64, :], c_ps[0:64, :], cb_t[0:64, 0:1], xn2_t[0:64, slA],
                        op0=OP.add, op1=OP.add,
                    )
                    # rows 64:128 read the residual from the shifted xn2 rows
                    nc.vector.scalar_tensor_tensor(
                        o_t[64:128, :], c_ps[64:128, :], cb_t[64:128, 0:1],
                        xn2_t[64:128, cB * 512 - 1:cB * 512 + 511],
                        op0=OP.add, op1=OP.add,
                    )
                    nc.sync.dma_start(out_d[:, slA], o_t[0:64, :])
                    nc.sync.dma_start(out_d[:, slB], o_t[64:128, :])

    nc.compile()
    return nc


def _prep_weights(W0, Ws, convW):
    w0r = W0.reshape(C, K, 2, 4 * H)
    w0p = np.zeros((2, 4, 128, 512), np.float32)
    for d in range(2):
        for kp in range(4):
            w0p[d, kp, 0:64] = w0r[:, 2 * kp, d]
            w0p[d, kp, 64:128] = w0r[:, 2 * kp + 1, d]
    wip = np.zeros((3, 2, 2, 128, 512), np.float32)
    for i in range(3):
        for d in range(2):
            for ct in range(2):
                wip[i, d, ct] = Ws[i][ct * 128:(ct + 1) * 128, d]
    # negate the f-gate output chunk so sigmoid(psum - bf) = 1-f = g
    # without a scale=-1 activation.
    w0p[:, :, :, 128:256] *= -1.0
    wip[:, :, :, :, 128:256] *= -1.0
    # layer-0 weights as fp8 DoubleRow pairs, scaled by W0S to stay out of
    # the e4m3 subnormal range (compensated by psum scale 1/W0S on-chip):
    # w0f8[d, ctp, p, ko, m] = W0S * w0p[d, 2*ctp+ko, p, m]
    w0f8 = np.zeros((2, 2, 128, 2, 512), np.float32)
    for ctp in range(2):
        for ko in range(2):
            w0f8[:, ctp, :, ko, :] = W0S * w0p[:, 2 * ctp + ko].transpose(0, 1, 2)
    cwp = np.zeros((2, 8, 128, C), np.float32)
    for ct in range(2):
        for k in range(8):
            cwp[ct, k] = convW[ct * 128:(ct + 1) * 128, :, k]
    bf16 = ml_dtypes.bfloat16
    f8 = ml_dtypes.float8_e4m3
    return w0f8.astype(f8), wip.astype(bf16), cwp.astype(bf16)


def kernel(**inputs):
    inputs = {k: np.asarray(v) for k, v in inputs.items()}
    x = inputs["x"].astype(np.float32)
    xs = np.ascontiguousarray(
        x.transpose(0, 2, 1, 3).reshape(B * T, C, F_)
    )  # (512, C, F)

    w0f8, wip, cwp = _prep_weights(
        inputs["W0"].astype(np.float32),
        [inputs[f"W{i}"].astype(np.float32) for i in (1, 2, 3)],
        inputs["convW"].astype(np.float32),
    )
    bfp = -np.stack([inputs[f"bf{i}"] for i in range(4)]).astype(np.float32)
    brp = np.stack([inputs[f"br{i}"] for i in range(4)]).astype(np.float32)
    gm = inputs["gamma"].reshape(C).astype(np.float32)
    bt = inputs["beta"].reshape(C).astype(np.float32)
    cb = np.tile(inputs["convb"].reshape(C).astype(np.float32), 2)  # [128]

    if "nc" not in _CACHE:
        _CACHE["nc"] = _build()
    nc = _CACHE["nc"]

    bf16 = ml_dtypes.bfloat16
    shared = {"w0f8": w0f8, "wip": wip, "cwp": cwp, "bfp": bfp, "brp": brp,
              "gm": gm, "bt": bt, "cb": cb}
    in_maps = []
    for core in range(NCORES):
        sh = xs[core * NLOC:(core + 1) * NLOC]  # (NLOC, C, F)
        u = np.ascontiguousarray(sh.transpose(1, 0, 2)).astype(bf16)  # (C, NLOC, F)
        in_maps.append({"u": u, **shared})

    trace = bool(os.environ.get("KBENCH_TRACE"))
    res = bass_utils.run_bass_kernel_spmd(
        nc, in_maps, list(range(NCORES)), trace=trace,
        tmpdir=os.environ.get("KBENCH_TMPDIR"),
    )
    _CACHE["last_result"] = res

    full = np.concatenate(
        [res.results[i]["o"].reshape(C, NLOC, F_) for i in range(NCORES)], axis=1
    )  # (C, 512, F)
    out = full.transpose(1, 0, 2).reshape(B, T, C, F_).transpose(0, 2, 1, 3)
    return np.ascontiguousarray(out.astype(np.float32))
